# revision 1
# baseline (speedup 1.0000x reference)
"""Multi-head causal attention (B=2, T=2048, E=1024, H=16, D=64) on 8 TRN2 cores.

Sharding: tensor-parallel over heads. Core c owns heads {2c, 2c+1} for both
batches. Each core computes its heads' q/k/v projections, causal attention,
and a partial output projection z_c = out_c @ Wo[:, 128c:128c+128].T.
Host combines: z = sum_c z_c + bo.

Note the reference computes wei = K @ Q^T, i.e. output token t attends over
s <= t with logits k_t . q_s. We compute ST[s, t] = q_s . k_t (s on
partitions) so that the A@V matmul needs no transposes, and get the softmax
denominator via a ones-column appended to V.
"""

import numpy as np
import ml_dtypes

import concourse.bacc as bacc
import concourse.mybir as mybir
import concourse.tile as tile
from concourse.bass_utils import run_bass_kernel_spmd
from concourse.masks import make_identity


def _make_runner(nc):
    """Persistent jitted SPMD callable (avoids per-call jit re-trace)."""
    import jax
    from jax.sharding import Mesh, NamedSharding, PartitionSpec
    try:
        from jax.experimental.shard_map import shard_map
    except ImportError:
        shard_map = jax.shard_map
    from concourse.bass2jax import (_bass_exec_p, install_neuronx_cc_hook,
                                    partition_id_tensor)

    install_neuronx_cc_hook()
    partition_name = (nc.partition_id_tensor.name
                      if nc.partition_id_tensor else None)
    in_names, out_names, out_avals, zero_outs = [], [], [], []
    for alloc in nc.m.functions[0].allocations:
        if not isinstance(alloc, mybir.MemoryLocationSet):
            continue
        name = alloc.memorylocations[0].name
        if alloc.kind == "ExternalInput":
            if name != partition_name:
                in_names.append(name)
        elif alloc.kind == "ExternalOutput":
            shape = tuple(alloc.tensor_shape)
            dtype = mybir.dt.np(alloc.dtype)
            out_names.append(name)
            out_avals.append(jax.core.ShapedArray(shape, dtype))
            zero_outs.append(np.zeros(shape, dtype))
    n_params = len(in_names)
    all_in = list(in_names) + list(out_names)
    if partition_name is not None:
        all_in.append(partition_name)

    def _body(*args):
        operands = list(args)
        if partition_name is not None:
            operands.append(partition_id_tensor())
        return tuple(_bass_exec_p.bind(
            *operands, out_avals=tuple(out_avals), in_names=tuple(all_in),
            out_names=tuple(out_names), lowering_input_output_aliases=(),
            sim_require_finite=True, sim_require_nnan=True, nc=nc))

    devices = jax.devices()[:N_CORES]
    mesh = Mesh(np.asarray(devices), ("core",))
    spec = NamedSharding(mesh, PartitionSpec("core"))
    rspec = NamedSharding(mesh, PartitionSpec())
    # inputs identical on every core are sent once and replicated
    replicated = {"xt", "mask"}
    in_specs = tuple(
        (PartitionSpec() if nm in replicated else PartitionSpec("core"))
        for nm in in_names) + (PartitionSpec("core"),) * len(out_names)
    fn = jax.jit(
        shard_map(_body, mesh=mesh, in_specs=in_specs,
                  out_specs=(PartitionSpec("core"),) * len(out_names),
                  check_rep=False),
        keep_unused=True)
    zeros_dev = [
        jax.device_put(np.zeros((N_CORES * z.shape[0], *z.shape[1:]), z.dtype),
                       spec) for z in zero_outs
    ]

    def run(in_maps):
        concat = [
            jax.device_put(np.asarray(in_maps[0][nm]), rspec)
            if nm in replicated else
            jax.device_put(
                np.concatenate([np.asarray(in_maps[c][nm])
                                for c in range(N_CORES)], axis=0), spec)
            for nm in in_names
        ]
        outs = fn(*concat, *zeros_dev)
        fulls = [np.asarray(outs[i]).reshape(N_CORES, *out_avals[i].shape)
                 for i in range(len(out_names))]
        return [{nm: fulls[i][c] for i, nm in enumerate(out_names)}
                for c in range(N_CORES)]

    return run

N_CORES = 8
B, T, E = 2, 2048, 1024
H, D = 16, 64
HPC = H // N_CORES          # heads per core = 2
F = HPC * D                 # local feature cols = 128
TBLK = 512                  # t-block width for stage A
NTB = T // TBLK             # 4
NSC = T // 128              # s-chunks = 16
NEC = E // 128              # e-chunks = 8
EXP_BIAS = -2.0             # exp(S + EXP_BIAS); cancels in softmax, guards overflow

F32 = mybir.dt.float32
F16 = mybir.dt.float16
F32R = mybir.dt.float32r
BF16 = mybir.dt.bfloat16
EXP = mybir.ActivationFunctionType.Exp


def build_nc(rep=1, cfg=None):
    cfg = dict(cfg or {})
    if cfg.get("bh"):
        return build_nc_bh(rep, cfg)
    any_copy = cfg.get("any_copy", False)
    evict = cfg.get("evict", "mixed")  # mixed|zscalar|zvector
    sp_bufs = cfg.get("sp_bufs", 2)
    op_bufs = cfg.get("op_bufs", 2)
    misc_bufs = cfg.get("misc_bufs", None)  # if set, tp+zp merged [128,512] x misc_bufs
    pt_bufs = cfg.get("pt_bufs", 4)
    out_q = cfg.get("out_q", "scalar")  # engine for output DMAs
    xt_bf16 = cfg.get("xt_bf16", False)
    s_bf16 = cfg.get("s_bf16", False)
    z_bf16 = cfg.get("z_bf16", False)
    skip_z = cfg.get("skip_z", False)
    skip_b = cfg.get("skip_b", False)
    skip_attn = cfg.get("skip_attn", False)
    td_form = cfg.get("td_form", False)
    sp_wide = cfg.get("sp_wide", False)
    nc = bacc.Bacc("TRN2", target_bir_lowering=False, debug=False,
                   num_devices=N_CORES)

    xt = nc.dram_tensor("xt", [B, E, T], BF16 if xt_bf16 else F32R,
                        kind="ExternalInput").ap()
    wq = nc.dram_tensor("wq", [E, F], BF16 if xt_bf16 else F32R, kind="ExternalInput").ap()
    wk = nc.dram_tensor("wk", [E, F], BF16 if xt_bf16 else F32R, kind="ExternalInput").ap()
    wv = nc.dram_tensor("wv", [E, F], BF16 if xt_bf16 else F32R, kind="ExternalInput").ap()
    wot = nc.dram_tensor("wot", [F, E], BF16 if z_bf16 else F32R,
                         kind="ExternalInput").ap()
    mask = nc.dram_tensor("mask", [128, 128], BF16, kind="ExternalInput").ap()
    zp = nc.dram_tensor("zp", [B, T, E], F16, kind="ExternalOutput").ap()

    with tile.TileContext(nc) as tc:
        with (
            tc.tile_pool(name="const", bufs=1) as cpool,
            tc.tile_pool(name="xtp", bufs=36) as xtp,
            tc.tile_pool(name="proj", bufs=2) as projp,
            tc.tile_pool(name="v2p", bufs=2 * NSC) as v2p,
            tc.tile_pool(name="ptp", bufs=pt_bufs) as ptp,
            tc.tile_pool(name="smallp", bufs=4) as smallp,
            tc.tile_pool(name="zsbp", bufs=3) as zsbp,
            tc.tile_pool(name="ps_s", bufs=sp_bufs, space="PSUM") as ps_s,
            tc.tile_pool(name="ps_o", bufs=op_bufs, space="PSUM") as ps_o,
            tc.tile_pool(name="ps_t", bufs=(misc_bufs or 2), space="PSUM") as ps_t,
        ):
            # ---- constants (loaded once) ----
            v_bf16 = cfg.get("v_bf16", False)
            ident = cpool.tile([128, 128], BF16 if v_bf16 else F32,
                               tag="ident")
            make_identity(nc, ident[:])
            mask_sb = cpool.tile([128, 128], BF16, tag="mask")
            nc.scalar.dma_start(mask_sb[:], mask)
            ebias = cpool.tile([128, 1], F32, tag="ebias")
            nc.vector.memset(ebias[:], EXP_BIAS)
            # one coalesced DMA per weight tensor: [E, F] -> [128, NEC*F]
            wq_sb = []
            wk_sb = []
            wv_sb = []
            for lst, wsrc, nm in ((wq_sb, wq, "wq"), (wk_sb, wk, "wk"),
                                  (wv_sb, wv, "wv")):
                t_ = cpool.tile([128, NEC * F], BF16 if xt_bf16 else F32R,
                                tag=f"wall{nm}")
                nc.scalar.dma_start(
                    t_.rearrange("p (a c) -> p a c", a=NEC),
                    wsrc.rearrange("(a p) c -> p a c", p=128))
                for e in range(NEC):
                    lst.append(t_[:, e * F:(e + 1) * F])
            wot_sb = cpool.tile([F, E], BF16 if z_bf16 else F32R, tag="wot")
            nc.scalar.dma_start(wot_sb[:], wot)

            def body():
                z_defer = cfg.get("z_defer", False)
                interleave = cfg.get("interleave", False)
                st = {"prev": None}

                def emit_xth(b, qt):
                    for e in range(NEC):
                        t_ = xtp.tile([128, T // 4],
                                      BF16 if xt_bf16 else F32R, tag="xt")
                        nc.sync.dma_start(
                            t_[:], xt[b, e * 128:(e + 1) * 128,
                                      qt * (T // 4):(qt + 1) * (T // 4)])
                        st["xth"][e][qt] = t_

                def emit_proj_chunk(nm, tp2):
                    wsb = {"q": wq_sb, "k": wk_sb, "v": wv_sb}[nm]
                    if tp2 == 0:
                        if nm == "v":
                            pdt = BF16 if v_bf16 else F32
                        else:
                            pdt = BF16 if s_bf16 else F32R
                        dstn = projp.tile([128, T], pdt, tag=f"{nm}T2")
                        st[nm] = dstn
                    dst = st[nm]
                    ps = ps_s.tile([128, 1024], F32, tag="sp")
                    for half in range(2):
                        c0 = tp2 * 1024 + half * 512
                        for e in range(NEC):
                            nc.tensor.matmul(
                                ps[:, half * 512:(half + 1) * 512],
                                wsb[e],
                                st["xth"][e][c0 // 512][:],
                                start=(e == 0), stop=(e == NEC - 1))
                    (nc.any if any_copy else nc.vector).tensor_copy(
                        dst[:, tp2 * 1024:(tp2 + 1) * 1024], ps[:])

                def emit_v2(s):
                    if v_bf16:
                        tpw = ps_t.tile([128, 512], BF16, tag="tp")
                    else:
                        tpw = ps_t.tile([128, 512], F32, tag="tp")
                    tp_ = tpw[:, 0:128]
                    nc.tensor.matmul(tp_[:], st["v"][:, s * 128:(s + 1) * 128],
                                     ident[:], is_transpose=True)
                    v2t = v2p.tile([128, 130], BF16, tag="v2")
                    v2r = v2t.rearrange("p (g c) -> p g c", g=2)
                    nc.vector.memset(v2r[:, :, 64:65], 1.0)
                    nc.vector.tensor_copy(
                        v2r[:, :, 0:64],
                        tp_.rearrange("p (g c) -> p g c", g=2))
                    st["v2"][s] = v2t

                def emit_z(outT, b, tb):
                    for jp2 in range(2):
                        zsb = zsbp.tile([128, 2048], F16, tag="zsb")
                        for jj in range(2):
                            j = 2 * jp2 + jj
                            for eb in range(2):
                                zps = ps_t.tile([128, 512], F32,
                                                tag="tp")
                                nc.tensor.matmul(
                                    zps[:],
                                    outT[:, j * 128:(j + 1) * 128],
                                    wot_sb[:, eb * 512:(eb + 1) * 512],
                                    start=True, stop=True)
                                dstsl = zsb[:, jj * 1024 + eb * 512:
                                            jj * 1024 + (eb + 1) * 512]
                                if evict == "zscalar":
                                    nc.scalar.copy(dstsl, zps[:])
                                elif evict == "zvector":
                                    nc.vector.tensor_copy(dstsl, zps[:])
                                elif any_copy:
                                    nc.any.tensor_copy(dstsl, zps[:])
                                elif eb == 0:
                                    nc.vector.tensor_copy(dstsl, zps[:])
                                else:
                                    nc.scalar.copy(dstsl, zps[:])
                        t0r = (4 * tb + 2 * jp2) * 128
                        getattr(nc, out_q).dma_start(
                            zp[b, t0r:t0r + 256, :]
                            .rearrange("(a p) c -> p a c", p=128),
                            zsb.rearrange("p (a c) -> p a c", a=2))

                def emit_stageB(b, tb):
                    qT2, kT2, vT2, v2 = st["q"], st["k"], st["v"], st["v2"]
                    slast = 4 * tb + 3
                    po = {}
                    for h in range(2 if not skip_b else 0):
                        po_t = ps_o.tile([65, 512], F32, tag="op")
                        po[h] = po_t
                    npairs = 2 * tb + 2
                    for p in range(npairs):
                        pts = []
                        for h in range(2):
                            ps = ps_s.tile([128, 1024], F32, tag="sp")
                            for dp in range(2):
                                si = 2 * p + dp
                                r = si - 4 * tb
                                rtrim = (1, 2, 3) if s_bf16 else (1, 2)
                                c0 = 128 * r if r in rtrim else 0
                                nc.tensor.matmul(
                                    ps[:, dp * 512 + c0:(dp + 1) * 512],
                                    qT2[64 * h:64 * h + 64,
                                        si * 128:(si + 1) * 128],
                                    kT2[64 * h:64 * h + 64,
                                        tb * 512 + c0:(tb + 1) * 512],
                                    start=True, stop=True)
                            pt = ptp.tile([128, 1024], BF16, tag="pt")
                            nc.scalar.activation(pt[:], ps[:], EXP,
                                                 bias=ebias[:])
                            for dp in range(2):
                                si = 2 * p + dp
                                r = si - 4 * tb
                                if 0 <= r < 4:
                                    sl = pt[:, dp * 512 + r * 128:
                                            dp * 512 + (r + 1) * 128]
                                    meng = (nc.gpsimd if cfg.get("mask_pool")
                                            else nc.vector)
                                    meng.tensor_mul(sl, sl, mask_sb[:])
                            pts.append(pt)
                        for dp in range(2 if not skip_b else 0):
                            si = 2 * p + dp
                            for h in range(2):
                                r = si - 4 * tb
                                c0 = max(r, 0) * 128
                                nc.tensor.matmul(
                                    po[h][:, c0:512],
                                    v2[si][:, h * 65:(h + 1) * 65],
                                    pts[h][:, dp * 512 + c0:
                                           (dp + 1) * 512],
                                    start=(si == 0), stop=(si == slast),
                                    skip_group_check=True)
                        if z_defer and p == 0 and st["prev"] is not None:
                            emit_z(*st.pop("prev"))
                            st["prev"] = None

                    # ---- normalize (rows 1:65 / row 0) + partial z ----
                    if not skip_b and not skip_z:
                        outT = smallp.tile([128, 512],
                                           BF16 if z_bf16 else F32R,
                                           tag="outT")
                        for h in range(2):
                            rrow = smallp.tile([1, 512], F32, tag="rrow")
                            nc.vector.reciprocal(rrow[:], po[h][64:65, :])
                            rbc = smallp.tile([64, 512], F32, tag="rbc")
                            nc.gpsimd.partition_broadcast(rbc[:], rrow[:])
                            nc.vector.tensor_mul(
                                outT[64 * h:64 * h + 64, :],
                                po[h][0:64, :], rbc[:])
                        if z_defer:
                            st["prev"] = (outT, b, tb)
                        else:
                            emit_z(outT, b, tb)

                for b in range(B):
                    st["xth"] = [[None] * 4 for _ in range(NEC)]
                    st["v2"] = [None] * NSC
                    if interleave:
                        for tp2 in range(2):
                            for qt in (2 * tp2, 2 * tp2 + 1):
                                emit_xth(b, qt)
                            for nm in ("q", "k", "v"):
                                emit_proj_chunk(nm, tp2)
                            for s in range(8 * tp2, 8 * tp2 + 8):
                                emit_v2(s)
                            if not skip_attn:
                                emit_stageB(b, 2 * tp2)
                                emit_stageB(b, 2 * tp2 + 1)
                    else:
                        for qt in range(4):
                            emit_xth(b, qt)
                        for nm in ("q", "k", "v"):
                            for tp2 in range(2):
                                emit_proj_chunk(nm, tp2)
                        for s in range(NSC):
                            emit_v2(s)
                        for tb in range(NTB if not skip_attn else 0):
                            emit_stageB(b, tb)
                if z_defer and st["prev"] is not None:
                    emit_z(*st["prev"])

            if rep == 1:
                body()
            elif cfg.get("unroll"):
                for _ in range(rep):
                    body()
            elif cfg.get("sreset"):
                with tc.For_i(0, rep, 1, staggered_reset=True):
                    body()
            else:
                with tc.For_i(0, rep, 1):
                    body()

    nc.compile()
    return nc


def build_nc_bh(rep=1, cfg=None):
    """Batch x head sharding: core c owns batch c//4 and heads
    4*(c%4)..4*(c%4)+3 (two pairs hp=0,1). All-bf16 matmul path.
    z partial accumulates both pairs in PSUM; host sums 4 cores/batch."""
    cfg = dict(cfg or {})
    sp_bufs = cfg.get("sp_bufs", 2)
    op_bufs = cfg.get("op_bufs", 2)
    pt_bufs = cfg.get("pt_bufs", 4)
    out_q = cfg.get("out_q", "sync")
    exp_trim = cfg.get("exp_trim", True)
    zevict = cfg.get("zevict", "any")  # any|vector|scalar|pool
    NHP = 2                     # head pairs per core
    nc = bacc.Bacc("TRN2", target_bir_lowering=False, debug=False,
                   num_devices=N_CORES)

    xt = nc.dram_tensor("xt", [E, T], BF16, kind="ExternalInput").ap()
    wq = nc.dram_tensor("wq", [E, NHP * F], BF16, kind="ExternalInput").ap()
    wk = nc.dram_tensor("wk", [E, NHP * F], BF16, kind="ExternalInput").ap()
    wv = nc.dram_tensor("wv", [E, NHP * F], BF16, kind="ExternalInput").ap()
    wot = nc.dram_tensor("wot", [NHP * F, E], BF16,
                         kind="ExternalInput").ap()
    mask = nc.dram_tensor("mask", [128, 128], BF16, kind="ExternalInput").ap()
    zp = nc.dram_tensor("zp", [T, E], F16, kind="ExternalOutput").ap()

    with tile.TileContext(nc) as tc:
        with (
            tc.tile_pool(name="const", bufs=1) as cpool,
            tc.tile_pool(name="xtp", bufs=36) as xtp,
            tc.tile_pool(name="proj", bufs=2) as projp,
            tc.tile_pool(name="v2p", bufs=2 * NSC + 8) as v2p,
            tc.tile_pool(name="ptp", bufs=pt_bufs) as ptp,
            tc.tile_pool(name="outTp",
                         bufs=(5 if cfg.get("z_defer") else 3)) as outTp,
            tc.tile_pool(name="smallp", bufs=4) as smallp,
            tc.tile_pool(name="zsbp", bufs=3) as zsbp,
            tc.tile_pool(name="ps_s", bufs=sp_bufs, space="PSUM") as ps_s,
            tc.tile_pool(name="ps_o", bufs=op_bufs, space="PSUM") as ps_o,
            tc.tile_pool(name="ps_t", bufs=2, space="PSUM") as ps_t,
        ):
            # ---- constants (loaded once) ----
            ident = cpool.tile([128, 128], F32, tag="ident")
            make_identity(nc, ident[:])
            mask_sb = cpool.tile([128, 128], BF16, tag="mask")
            nc.scalar.dma_start(mask_sb[:], mask)
            ebias = cpool.tile([128, 1], F32, tag="ebias")
            nc.vector.memset(ebias[:], EXP_BIAS)
            # weights: [E, 2F] -> [128, NEC*2F]; w[hp][e] = [128, F]
            wsb = {}
            for wsrc, nm in ((wq, "wq"), (wk, "wk"), (wv, "wv")):
                t_ = cpool.tile([128, NEC * NHP * F], BF16, tag=f"wall{nm}")
                nc.scalar.dma_start(
                    t_.rearrange("p (a c) -> p a c", a=NEC),
                    wsrc.rearrange("(a p) c -> p a c", p=128))
                wsb[nm] = [[t_[:, e * NHP * F + hp * F:
                               e * NHP * F + (hp + 1) * F]
                            for e in range(NEC)] for hp in range(NHP)]
            wot_sb = []
            for hp in range(NHP):
                t_ = cpool.tile([F, E], BF16, tag=f"wot{hp}")
                nc.scalar.dma_start(t_[:], wot[hp * F:(hp + 1) * F, :])
                wot_sb.append(t_)

            def body():
                xth = [[None] * 4 for _ in range(NEC)]
                qT2, kT2, vT2, v2 = {}, {}, {}, {hp: [None] * NSC
                                                 for hp in range(NHP)}
                z_defer = cfg.get("z_defer", False)
                interleave = cfg.get("interleave", False)

                def emit_xth(qt):
                    for e in range(NEC):
                        t_ = xtp.tile([128, T // 4], BF16, tag="xt")
                        eng = ((nc.sync, nc.scalar)[e % 2]
                               if cfg.get("xt_q") else nc.sync)
                        eng.dma_start(
                            t_[:], xt[e * 128:(e + 1) * 128,
                                      qt * (T // 4):(qt + 1) * (T // 4)])
                        xth[e][qt] = t_

                def emit_proj_chunk(hp, nm, tp2):
                    if tp2 == 0:
                        pdt = F32 if nm == "v" else BF16
                        dst = projp.tile([128, T], pdt, tag=f"{nm}T2_{hp}")
                        {"q": qT2, "k": kT2, "v": vT2}[nm][hp] = dst
                    dst = {"q": qT2, "k": kT2, "v": vT2}[nm][hp]
                    ps = ps_s.tile([128, 1024], F32, tag="sp")
                    for half in range(2):
                        c0 = tp2 * 1024 + half * 512
                        for e in range(NEC):
                            nc.tensor.matmul(
                                ps[:, half * 512:(half + 1) * 512],
                                wsb["w" + nm][hp][e],
                                xth[e][c0 // 512][:],
                                start=(e == 0), stop=(e == NEC - 1))
                    nc.any.tensor_copy(
                        dst[:, tp2 * 1024:(tp2 + 1) * 1024], ps[:])

                def emit_v2(hp, s):
                    tpw = ps_t.tile([128, 512], F32, tag="tp")
                    tp_ = tpw[:, 0:128]
                    nc.tensor.matmul(tp_[:],
                                     vT2[hp][:, s * 128:(s + 1) * 128],
                                     ident[:], is_transpose=True)
                    v2t = v2p.tile([128, 130], BF16, tag="v2")
                    v2r = v2t.rearrange("p (g c) -> p g c", g=2)
                    nc.vector.memset(v2r[:, :, 64:65], 1.0)
                    nc.vector.tensor_copy(
                        v2r[:, :, 0:64],
                        tp_.rearrange("p (g c) -> p g c", g=2))
                    v2[hp][s] = v2t

                def emit_z(zoutT, ztb):
                    for jp2 in range(2):
                        zsb = zsbp.tile([128, 2048], F16, tag="zsb")
                        for jj in range(2):
                            j = 2 * jp2 + jj
                            zpsl = []
                            for eb in range(2):
                                zps = ps_t.tile([128, 512], F32, tag="tp")
                                zpsl.append(zps)
                            for hp in range(NHP):
                                for eb in range(2):
                                    nc.tensor.matmul(
                                        zpsl[eb][:],
                                        zoutT[hp][:, j * 128:(j + 1) * 128],
                                        wot_sb[hp][:, eb * 512:(eb + 1) * 512],
                                        start=(hp == 0), stop=(hp == NHP - 1))
                            for eb in range(2):
                                dstsl = zsb[:, jj * 1024 + eb * 512:
                                            jj * 1024 + (eb + 1) * 512]
                                if zevict == "vector":
                                    nc.vector.tensor_copy(dstsl, zpsl[eb][:])
                                elif zevict == "scalar":
                                    nc.scalar.copy(dstsl, zpsl[eb][:])
                                else:
                                    nc.any.tensor_copy(dstsl, zpsl[eb][:])
                        t0r = (4 * ztb + 2 * jp2) * 128
                        getattr(nc, out_q).dma_start(
                            zp[t0r:t0r + 256, :]
                            .rearrange("(a p) c -> p a c", p=128),
                            zsb.rearrange("p (a c) -> p a c", a=2))

                prev_outT = None

                def emit_stageB(tb):
                    nonlocal prev_outT
                    slast = 4 * tb + 3
                    outT = {}
                    for hp in range(NHP):
                        po = {}
                        for h in range(2):
                            po_t = ps_o.tile([65, 512], F32, tag="op")
                            po[h] = po_t
                        npairs = 2 * tb + 2
                        for p in range(npairs):
                            pts = []
                            for h in range(2):
                                ps = ps_s.tile([128, 1024], F32, tag="sp")
                                for dp in range(2):
                                    si = 2 * p + dp
                                    r = si - 4 * tb
                                    c0 = 128 * r if r in (1, 2, 3) else 0
                                    nc.tensor.matmul(
                                        ps[:, dp * 512 + c0:(dp + 1) * 512],
                                        qT2[hp][64 * h:64 * h + 64,
                                                si * 128:(si + 1) * 128],
                                        kT2[hp][64 * h:64 * h + 64,
                                                tb * 512 + c0:(tb + 1) * 512],
                                        start=True, stop=True)
                                pt = ptp.tile([128, 1024], BF16, tag="pt")
                                if exp_trim and 2 * p >= 4 * tb:
                                    # diagonal pair: exp live cols per half
                                    for dp in range(2):
                                        r = 2 * p + dp - 4 * tb
                                        c0 = 128 * r if r in (1, 2, 3) else 0
                                        nc.scalar.activation(
                                            pt[:, dp * 512 + c0:
                                               (dp + 1) * 512],
                                            ps[:, dp * 512 + c0:
                                               (dp + 1) * 512],
                                            EXP, bias=ebias[:])
                                else:
                                    nc.scalar.activation(pt[:], ps[:], EXP,
                                                         bias=ebias[:])
                                for dp in range(2):
                                    si = 2 * p + dp
                                    r = si - 4 * tb
                                    if 0 <= r < 4:
                                        sl = pt[:, dp * 512 + r * 128:
                                                dp * 512 + (r + 1) * 128]
                                        nc.vector.tensor_mul(sl, sl,
                                                             mask_sb[:])
                                pts.append(pt)
                            for dp in range(2):
                                si = 2 * p + dp
                                for h in range(2):
                                    r = si - 4 * tb
                                    c0 = max(r, 0) * 128
                                    nc.tensor.matmul(
                                        po[h][:, c0:512],
                                        v2[hp][si][:, h * 65:(h + 1) * 65],
                                        pts[h][:, dp * 512 + c0:
                                               (dp + 1) * 512],
                                        start=(si == 0), stop=(si == slast),
                                        skip_group_check=True)
                            # deferred z of the previous t-block slots into
                            # the PE queue here, after deps are long ready
                            if z_defer and hp == 0 and p == 0 and tb > 0:
                                emit_z(prev_outT, tb - 1)

                        # ---- normalize (rows 0:64 / row 64) ----
                        oT = outTp.tile([128, 512], BF16, tag="outT")
                        for h in range(2):
                            rrow = smallp.tile([1, 512], F32, tag="rrow")
                            nc.vector.reciprocal(rrow[:], po[h][64:65, :])
                            rbc = smallp.tile([64, 512], F32, tag="rbc")
                            nc.gpsimd.partition_broadcast(rbc[:], rrow[:])
                            nc.vector.tensor_mul(
                                oT[64 * h:64 * h + 64, :],
                                po[h][0:64, :], rbc[:])
                        outT[hp] = oT

                    # ---- z: accumulate both pairs in PSUM ----
                    if z_defer:
                        prev_outT = outT
                    else:
                        emit_z(outT, tb)

                if interleave:
                    for tp2 in range(2):
                        for qt in (2 * tp2, 2 * tp2 + 1):
                            emit_xth(qt)
                        for hp in range(NHP):
                            for nm in ("q", "k", "v"):
                                emit_proj_chunk(hp, nm, tp2)
                        for hp in range(NHP):
                            for s in range(8 * tp2, 8 * tp2 + 8):
                                emit_v2(hp, s)
                        emit_stageB(2 * tp2)
                        emit_stageB(2 * tp2 + 1)
                else:
                    for qt in range(4):
                        emit_xth(qt)
                    for hp in range(NHP):
                        for nm in ("q", "k", "v"):
                            for tp2 in range(2):
                                emit_proj_chunk(hp, nm, tp2)
                        for s in range(NSC):
                            emit_v2(hp, s)
                    for tb in range(NTB):
                        emit_stageB(tb)
                if z_defer and prev_outT is not None:
                    emit_z(prev_outT, NTB - 1)

            if rep == 1:
                body()
            elif cfg.get("unroll"):
                for _ in range(rep):
                    body()
            elif cfg.get("body2") and (rep - 1) % 2 == 0:
                with tc.For_i(0, (rep - 1) // 2, 1):
                    body()
                    body()
                body()
            elif cfg.get("sreset"):
                with tc.For_i(0, rep, 1, staggered_reset=True):
                    body()
            else:
                with tc.For_i(0, rep, 1):
                    body()

    nc.compile()
    return nc


def make_in_maps_bh(inputs, Wk, Wq, Wv, Wo):
    """Shard: core c gets batch c//4, heads 4*(c%4)..4*(c%4)+3."""
    bf = ml_dtypes.bfloat16
    scale = np.float32(D ** -0.5)
    tri = (np.arange(128)[None, :] >= np.arange(128)[:, None])
    mask = tri.astype(bf)
    in_maps = []
    for c in range(N_CORES):
        b = c // 4
        h0 = 4 * (c % 4)
        xt = np.ascontiguousarray(inputs[b].T).astype(bf)
        wq2 = np.concatenate([Wq[h0 + i] for i in range(4)], axis=1)
        wk2 = np.concatenate([Wk[h0 + i] for i in range(4)], axis=1) * scale
        wv2 = np.concatenate([Wv[h0 + i] for i in range(4)], axis=1)
        wot = np.ascontiguousarray(Wo[:, 64 * h0:64 * (h0 + 4)].T)
        in_maps.append({
            "xt": xt,
            "wq": np.ascontiguousarray(wq2).astype(bf),
            "wk": np.ascontiguousarray(wk2).astype(bf),
            "wv": np.ascontiguousarray(wv2).astype(bf),
            "wot": wot.astype(bf),
            "mask": mask,
        })
    return in_maps


def make_in_maps(inputs, Wk, Wq, Wv, Wo, xt_bf16=False, z_bf16=False):
    """Shard full inputs into per-core input maps."""
    wdt = ml_dtypes.bfloat16 if xt_bf16 else np.float32
    zdt = ml_dtypes.bfloat16 if z_bf16 else np.float32
    xt = np.ascontiguousarray(inputs.transpose(0, 2, 1)).astype(wdt)
    scale = np.float32(D ** -0.5)
    tri = (np.arange(128)[None, :] >= np.arange(128)[:, None])
    mask = tri.astype(ml_dtypes.bfloat16)
    in_maps = []
    for c in range(N_CORES):
        h0 = HPC * c
        wq2 = np.ascontiguousarray(
            np.concatenate([Wq[h0 + i] for i in range(HPC)], axis=1))
        wk2 = np.ascontiguousarray(
            np.concatenate([Wk[h0 + i] for i in range(HPC)], axis=1)) * scale
        wv2 = np.ascontiguousarray(
            np.concatenate([Wv[h0 + i] for i in range(HPC)], axis=1))
        wot = np.ascontiguousarray(Wo[:, F * c:F * (c + 1)].T)
        in_maps.append({
            "xt": xt,
            "wq": wq2.astype(wdt),
            "wk": wk2.astype(wdt),
            "wv": wv2.astype(wdt),
            "wot": wot.astype(zdt),
            "mask": mask,
        })
    return in_maps


_NC = None
_RUN = None
DEFAULT_CFG = {"any_copy": True, "out_q": "sync", "xt_bf16": True,
               "s_bf16": True, "z_bf16": True}


def combine(zp_list, bo, cfg):
    """Combine per-core zp partials into the full [B, T, E] output."""
    z = np.zeros((B, T, E), dtype=np.float32)
    if cfg.get("bh"):
        for c in range(N_CORES):
            z[c // 4] += np.asarray(zp_list[c]).astype(np.float32)
    else:
        for c in range(N_CORES):
            z += np.asarray(zp_list[c]).astype(np.float32)
    return z + bo.astype(np.float32)


def kernel(inputs, Wk, Wq, Wv, Wo, bo):
    global _NC, _RUN
    if _NC is None:
        _NC = build_nc(cfg=DEFAULT_CFG)
    if DEFAULT_CFG.get("bh"):
        in_maps = make_in_maps_bh(inputs, Wk, Wq, Wv, Wo)
    else:
        in_maps = make_in_maps(inputs, Wk, Wq, Wv, Wo,
                               xt_bf16=DEFAULT_CFG.get("xt_bf16", False),
                               z_bf16=DEFAULT_CFG.get("z_bf16", False))
    try:
        if _RUN is None:
            _RUN = _make_runner(_NC)
        results = _RUN(in_maps)
    except Exception:
        _RUN = False if _RUN is None else _RUN
        res = run_bass_kernel_spmd(_NC, in_maps,
                                   core_ids=list(range(N_CORES)))
        results = res.results
    return combine([results[c]["zp"] for c in range(N_CORES)], bo,
                   DEFAULT_CFG)



# revision 29
# speedup vs baseline: 1.1252x; 1.1252x over previous
"""Multi-head causal attention (B=2, T=2048, E=1024, H=16, D=64) on 8 TRN2 cores.

Sharding: tensor-parallel over heads. Core c owns heads {2c, 2c+1} for both
batches. Each core computes its heads' q/k/v projections, causal attention,
and a partial output projection z_c = out_c @ Wo[:, 128c:128c+128].T.
Host combines: z = sum_c z_c + bo.

Note the reference computes wei = K @ Q^T, i.e. output token t attends over
s <= t with logits k_t . q_s. We compute ST[s, t] = q_s . k_t (s on
partitions) so that the A@V matmul needs no transposes, and get the softmax
denominator via a ones-column appended to V.
"""

import numpy as np
import ml_dtypes

import concourse.bacc as bacc
import concourse.mybir as mybir
import concourse.tile as tile
from concourse.bass_utils import run_bass_kernel_spmd
from concourse.masks import make_identity


def _make_runner(nc):
    """Persistent jitted SPMD callable (avoids per-call jit re-trace)."""
    import jax
    from jax.sharding import Mesh, NamedSharding, PartitionSpec
    try:
        from jax.experimental.shard_map import shard_map
    except ImportError:
        shard_map = jax.shard_map
    from concourse.bass2jax import (_bass_exec_p, install_neuronx_cc_hook,
                                    partition_id_tensor)

    install_neuronx_cc_hook()
    partition_name = (nc.partition_id_tensor.name
                      if nc.partition_id_tensor else None)
    in_names, out_names, out_avals, zero_outs = [], [], [], []
    for alloc in nc.m.functions[0].allocations:
        if not isinstance(alloc, mybir.MemoryLocationSet):
            continue
        name = alloc.memorylocations[0].name
        if alloc.kind == "ExternalInput":
            if name != partition_name:
                in_names.append(name)
        elif alloc.kind == "ExternalOutput":
            shape = tuple(alloc.tensor_shape)
            dtype = mybir.dt.np(alloc.dtype)
            out_names.append(name)
            out_avals.append(jax.core.ShapedArray(shape, dtype))
            zero_outs.append(np.zeros(shape, dtype))
    n_params = len(in_names)
    all_in = list(in_names) + list(out_names)
    if partition_name is not None:
        all_in.append(partition_name)

    def _body(*args):
        operands = list(args)
        if partition_name is not None:
            operands.append(partition_id_tensor())
        return tuple(_bass_exec_p.bind(
            *operands, out_avals=tuple(out_avals), in_names=tuple(all_in),
            out_names=tuple(out_names), lowering_input_output_aliases=(),
            sim_require_finite=True, sim_require_nnan=True, nc=nc))

    devices = jax.devices()[:N_CORES]
    mesh = Mesh(np.asarray(devices), ("core",))
    spec = NamedSharding(mesh, PartitionSpec("core"))
    rspec = NamedSharding(mesh, PartitionSpec())
    # inputs identical on every core are sent once and replicated
    replicated = {"xt", "mask", "ntri"}
    in_specs = tuple(
        (PartitionSpec() if nm in replicated else PartitionSpec("core"))
        for nm in in_names) + (PartitionSpec("core"),) * len(out_names)
    fn = jax.jit(
        shard_map(_body, mesh=mesh, in_specs=in_specs,
                  out_specs=(PartitionSpec("core"),) * len(out_names),
                  check_rep=False),
        keep_unused=True)
    zeros_dev = [
        jax.device_put(np.zeros((N_CORES * z.shape[0], *z.shape[1:]), z.dtype),
                       spec) for z in zero_outs
    ]

    def run(in_maps):
        concat = [
            jax.device_put(np.asarray(in_maps[0][nm]), rspec)
            if nm in replicated else
            jax.device_put(
                np.concatenate([np.asarray(in_maps[c][nm])
                                for c in range(N_CORES)], axis=0), spec)
            for nm in in_names
        ]
        outs = fn(*concat, *zeros_dev)
        fulls = [np.asarray(outs[i]).reshape(N_CORES, *out_avals[i].shape)
                 for i in range(len(out_names))]
        return [{nm: fulls[i][c] for i, nm in enumerate(out_names)}
                for c in range(N_CORES)]

    return run

N_CORES = 8
B, T, E = 2, 2048, 1024
H, D = 16, 64
HPC = H // N_CORES          # heads per core = 2
F = HPC * D                 # local feature cols = 128
TBLK = 512                  # t-block width for stage A
NTB = T // TBLK             # 4
NSC = T // 128              # s-chunks = 16
NEC = E // 128              # e-chunks = 8
EXP_BIAS = -2.0             # exp(S + EXP_BIAS); cancels in softmax, guards overflow

F32 = mybir.dt.float32
F16 = mybir.dt.float16
F32R = mybir.dt.float32r
BF16 = mybir.dt.bfloat16
EXP = mybir.ActivationFunctionType.Exp


def build_nc(rep=1, cfg=None):
    cfg = dict(cfg or {})
    if cfg.get("bh"):
        return build_nc_bh(rep, cfg)
    any_copy = cfg.get("any_copy", False)
    evict = cfg.get("evict", "mixed")  # mixed|zscalar|zvector
    sp_bufs = cfg.get("sp_bufs", 2)
    op_bufs = cfg.get("op_bufs", 2)
    misc_bufs = cfg.get("misc_bufs", None)  # if set, tp+zp merged [128,512] x misc_bufs
    pt_bufs = cfg.get("pt_bufs", 4)
    out_q = cfg.get("out_q", "scalar")  # engine for output DMAs
    xt_bf16 = cfg.get("xt_bf16", False)
    s_bf16 = cfg.get("s_bf16", False)
    z_bf16 = cfg.get("z_bf16", False)
    skip_z = cfg.get("skip_z", False)
    skip_b = cfg.get("skip_b", False)
    skip_attn = cfg.get("skip_attn", False)
    td_form = cfg.get("td_form", False)
    sp_wide = cfg.get("sp_wide", False)
    nc = bacc.Bacc("TRN2", target_bir_lowering=False, debug=False,
                   num_devices=N_CORES)

    xt = nc.dram_tensor("xt", [B, E, T], BF16 if xt_bf16 else F32R,
                        kind="ExternalInput").ap()
    wq = nc.dram_tensor("wq", [E, F], BF16 if xt_bf16 else F32R, kind="ExternalInput").ap()
    wk = nc.dram_tensor("wk", [E, F], BF16 if xt_bf16 else F32R, kind="ExternalInput").ap()
    wv = nc.dram_tensor("wv", [E, F], BF16 if xt_bf16 else F32R, kind="ExternalInput").ap()
    wot = nc.dram_tensor("wot", [F, E], BF16 if z_bf16 else F32R,
                         kind="ExternalInput").ap()
    mask = nc.dram_tensor("mask", [128, 128], BF16, kind="ExternalInput").ap()
    if cfg.get("mask_mm"):
        ntri = nc.dram_tensor("ntri", [128, 128], BF16,
                              kind="ExternalInput").ap()
    zp = nc.dram_tensor("zp", [B, T, E], F16, kind="ExternalOutput").ap()

    with tile.TileContext(nc) as tc:
        with (
            tc.tile_pool(name="const", bufs=1) as cpool,
            tc.tile_pool(name="xtp", bufs=cfg.get("xtp_bufs", 36)) as xtp,
            tc.tile_pool(name="proj", bufs=2) as projp,
            tc.tile_pool(name="v2p", bufs=2 * NSC) as v2p,
            tc.tile_pool(name="ptp", bufs=pt_bufs) as ptp,
            tc.tile_pool(name="smallp", bufs=4) as smallp,
            tc.tile_pool(name="zsbp", bufs=3) as zsbp,
            tc.tile_pool(name="ps_s", bufs=sp_bufs, space="PSUM") as ps_s,
            tc.tile_pool(name="ps_o", bufs=op_bufs, space="PSUM") as ps_o,
            tc.tile_pool(name="ps_t", bufs=(misc_bufs or 2), space="PSUM") as ps_t,
        ):
            # ---- constants (loaded once) ----
            v_bf16 = cfg.get("v_bf16", False)
            ident = cpool.tile([128, 128], BF16 if v_bf16 else F32,
                               tag="ident")
            make_identity(nc, ident[:])
            mask_sb = cpool.tile([128, 128], BF16, tag="mask")
            nc.scalar.dma_start(mask_sb[:], mask)
            ebias = cpool.tile([128, 1], F32, tag="ebias")
            nc.vector.memset(ebias[:], EXP_BIAS)
            if cfg.get("mask_mm"):
                # -30 * strict-lower-tri constant and bf16 identity: the
                # causal mask is accumulated into S by one extra matmul
                # (ident.T @ ntri) instead of a DVE multiply after exp.
                ntri_sb = cpool.tile([128, 128], BF16, tag="ntri")
                nc.scalar.dma_start(ntri_sb[:], ntri)
                identb = cpool.tile([128, 128], BF16, tag="identb")
                make_identity(nc, identb[:])
            # one coalesced DMA per weight tensor: [E, F] -> [128, NEC*F]
            wq_sb = []
            wk_sb = []
            wv_sb = []
            for lst, wsrc, nm in ((wq_sb, wq, "wq"), (wk_sb, wk, "wk"),
                                  (wv_sb, wv, "wv")):
                t_ = cpool.tile([128, NEC * F], BF16 if xt_bf16 else F32R,
                                tag=f"wall{nm}")
                nc.scalar.dma_start(
                    t_.rearrange("p (a c) -> p a c", a=NEC),
                    wsrc.rearrange("(a p) c -> p a c", p=128))
                for e in range(NEC):
                    lst.append(t_[:, e * F:(e + 1) * F])
            wot_sb = cpool.tile([F, E], BF16 if z_bf16 else F32R, tag="wot")
            nc.scalar.dma_start(wot_sb[:], wot)

            def body():
                z_defer = cfg.get("z_defer", False)
                interleave = cfg.get("interleave", False)
                st = {"prev": None}

                def emit_xth(b, qt):
                    if cfg.get("xt_wide"):
                        # one [128, T] DMA per e-chunk (4KB/partition line),
                        # issued at qt==0; xth[e][qt] slices the wide tile
                        if qt != 0:
                            return
                        for e in range(NEC):
                            t_ = xtp.tile([128, T], BF16 if xt_bf16 else F32R,
                                          tag="xt", bufs=cfg.get(
                                              "xtw_bufs", 9))
                            eng = ((nc.sync, nc.gpsimd)[e % 2]
                                   if cfg.get("xt_q") else nc.sync)
                            eng.dma_start(
                                t_[:], xt[b, e * 128:(e + 1) * 128, :])
                            for q4 in range(4):
                                st["xth"][e][q4] = t_[:, q4 * (T // 4):
                                                      (q4 + 1) * (T // 4)]
                        return
                    for e in range(NEC):
                        t_ = xtp.tile([128, T // 4],
                                      BF16 if xt_bf16 else F32R, tag="xt")
                        eng = ((nc.sync, nc.gpsimd)[e % 2]
                               if cfg.get("xt_q") else nc.sync)
                        eng.dma_start(
                            t_[:], xt[b, e * 128:(e + 1) * 128,
                                      qt * (T // 4):(qt + 1) * (T // 4)])
                        st["xth"][e][qt] = t_

                def emit_proj_chunk(nm, tp2):
                    wsb = {"q": wq_sb, "k": wk_sb, "v": wv_sb}[nm]
                    if tp2 == 0:
                        if nm == "v":
                            pdt = BF16 if v_bf16 else F32
                        else:
                            pdt = BF16 if s_bf16 else F32R
                        dstn = projp.tile([128, T], pdt, tag=f"{nm}T2")
                        st[nm] = dstn
                    dst = st[nm]
                    if cfg.get("proj_tp"):
                        # proj accumulates in the tp pool (1-bank halves) so
                        # the sp pool stays dedicated to the S/exp pipeline
                        for half in range(2):
                            c0 = tp2 * 1024 + half * 512
                            ph = ps_t.tile([128, 512], F32, tag="tp",
                                           name="ph")
                            for e in range(NEC):
                                nc.tensor.matmul(
                                    ph[:], wsb[e],
                                    st["xth"][e][c0 // 512][:],
                                    start=(e == 0), stop=(e == NEC - 1))
                            (nc.any if any_copy else nc.vector).tensor_copy(
                                dst[:, c0:c0 + 512], ph[:])
                        return
                    ps = ps_s.tile([128, 1024], F32, tag="sp")
                    for half in range(2):
                        c0 = tp2 * 1024 + half * 512
                        for e in range(NEC):
                            nc.tensor.matmul(
                                ps[:, half * 512:(half + 1) * 512],
                                wsb[e],
                                st["xth"][e][c0 // 512][:],
                                start=(e == 0), stop=(e == NEC - 1))
                    (nc.any if any_copy else nc.vector).tensor_copy(
                        dst[:, tp2 * 1024:(tp2 + 1) * 1024], ps[:])

                def emit_v2(s):
                    if v_bf16:
                        tpw = ps_t.tile([128, 512], BF16, tag="tp")
                    else:
                        tpw = ps_t.tile([128, 512], F32, tag="tp")
                    tp_ = tpw[:, 0:128]
                    nc.tensor.matmul(tp_[:], st["v"][:, s * 128:(s + 1) * 128],
                                     ident[:], is_transpose=True)
                    v2t = v2p.tile([128, 130], BF16, tag="v2")
                    v2r = v2t.rearrange("p (g c) -> p g c", g=2)
                    nc.vector.memset(v2r[:, :, 64:65], 1.0)
                    nc.vector.tensor_copy(
                        v2r[:, :, 0:64],
                        tp_.rearrange("p (g c) -> p g c", g=2))
                    st["v2"][s] = v2t

                def emit_z(outT, b, tb):
                    for jp2 in range(2):
                        zsb = zsbp.tile([128, 2048], F16, tag="zsb")
                        for jj in range(2):
                            j = 2 * jp2 + jj
                            for eb in range(2):
                                zps = ps_t.tile([128, 512], F32,
                                                tag="tp")
                                nc.tensor.matmul(
                                    zps[:],
                                    outT[:, j * 128:(j + 1) * 128],
                                    wot_sb[:, eb * 512:(eb + 1) * 512],
                                    start=True, stop=True)
                                dstsl = zsb[:, jj * 1024 + eb * 512:
                                            jj * 1024 + (eb + 1) * 512]
                                if evict == "zscalar":
                                    nc.scalar.copy(dstsl, zps[:])
                                elif evict == "zvector":
                                    nc.vector.tensor_copy(dstsl, zps[:])
                                elif any_copy:
                                    nc.any.tensor_copy(dstsl, zps[:])
                                elif eb == 0:
                                    nc.vector.tensor_copy(dstsl, zps[:])
                                else:
                                    nc.scalar.copy(dstsl, zps[:])
                        t0r = (4 * tb + 2 * jp2) * 128
                        getattr(nc, out_q).dma_start(
                            zp[b, t0r:t0r + 256, :]
                            .rearrange("(a p) c -> p a c", p=128),
                            zsb.rearrange("p (a c) -> p a c", a=2))

                expctr = [0]

                def emit_exp(pt, ps):
                    """exp(ps + EXP_BIAS) -> pt; every k-th tile via DVE
                    Schraudolph approximation (cfg dve_exp = k)."""
                    k = cfg.get("dve_exp", 0)
                    expctr[0] += 1
                    if k and (expctr[0] % k == 0):
                        A = float(2.0 ** 23 / np.log(2.0))
                        Bc = 1065353216.0 - 366393.0 + A * EXP_BIAS
                        ti = ptp.tile([128, 1024], mybir.dt.int32,
                                      tag="ti", bufs=2, name="ti")
                        nc.vector.tensor_scalar(
                            ti[:], ps[:], A, Bc,
                            mybir.AluOpType.mult, mybir.AluOpType.add)
                        nc.vector.tensor_copy(pt[:], ti[:].bitcast(F32))
                    else:
                        nc.scalar.activation(pt[:], ps[:], EXP,
                                             bias=ebias[:])

                def emit_stageB_quad(b, tb):
                    """Quad variant: S in [128,2048] bf16 PSUM (2 banks), one
                    exp per quad, h0/h1 S mms interleaved for row-packing."""
                    qT2, kT2, v2 = st["q"], st["k"], st["v2"]
                    slast = 4 * tb + 3
                    po = {h: ps_o.tile([65, 512], F32, tag="op", name="po")
                          for h in range(2)}
                    nquads = tb + 1
                    for q in range(nquads):
                        psq = {h: ps_s.tile([128, 2048], BF16, tag="sp", name="psq")
                               for h in range(2)}
                        for dp in range(4):
                            si = 4 * q + dp
                            r = si - 4 * tb
                            c0 = 128 * r if r in (1, 2, 3) else 0
                            for h in range(2):
                                nc.tensor.matmul(
                                    psq[h][:, dp * 512 + c0:(dp + 1) * 512],
                                    qT2[64 * h:64 * h + 64,
                                        si * 128:(si + 1) * 128],
                                    kT2[64 * h:64 * h + 64,
                                        tb * 512 + c0:(tb + 1) * 512],
                                    start=True, stop=True)
                        pts = []
                        for h in range(2):
                            pt = ptp.tile([128, 2048], BF16, tag="pt")
                            nc.scalar.activation(pt[:], psq[h][:], EXP,
                                                 bias=ebias[:])
                            for dp in range(4):
                                r = 4 * q + dp - 4 * tb
                                if 0 <= r < 4:
                                    sl = pt[:, dp * 512 + r * 128:
                                            dp * 512 + (r + 1) * 128]
                                    meng = (nc.gpsimd if cfg.get("mask_pool")
                                            else nc.vector)
                                    meng.tensor_mul(sl, sl, mask_sb[:])
                            pts.append(pt)
                        for dp in range(4):
                            si = 4 * q + dp
                            r = si - 4 * tb
                            c0 = max(r, 0) * 128
                            for h in range(2):
                                nc.tensor.matmul(
                                    po[h][:, c0:512],
                                    v2[si][:, h * 65:(h + 1) * 65],
                                    pts[h][:, dp * 512 + c0:(dp + 1) * 512],
                                    start=(si == 0), stop=(si == slast),
                                    skip_group_check=True)
                        if (cfg.get("z_defer") and q == 0
                                and st["prev"] is not None):
                            emit_z(*st.pop("prev"))
                            st["prev"] = None

                    # ---- normalize (rows 1:65 / row 0) + partial z ----
                    if not skip_z:
                        outT = smallp.tile([128, 512],
                                           BF16 if z_bf16 else F32R,
                                           tag="outT")
                        for h in range(2):
                            rrow = smallp.tile([1, 512], F32, tag="rrow")
                            nc.vector.reciprocal(rrow[:], po[h][64:65, :])
                            rbc = smallp.tile([64, 512], F32, tag="rbc")
                            nc.gpsimd.partition_broadcast(rbc[:], rrow[:])
                            nc.vector.tensor_mul(
                                outT[64 * h:64 * h + 64, :],
                                po[h][0:64, :], rbc[:])
                        if cfg.get("z_defer"):
                            st["prev"] = (outT, b, tb)
                        else:
                            emit_z(outT, b, tb)

                def emit_stageB(b, tb):
                    if cfg.get("quad"):
                        return emit_stageB_quad(b, tb)
                    qT2, kT2, vT2, v2 = st["q"], st["k"], st["v"], st["v2"]
                    exp_trim = cfg.get("exp_trim", False)
                    slast = 4 * tb + 3
                    po = {}
                    for h in range(2 if not skip_b else 0):
                        po_t = ps_o.tile([65, 512], F32, tag="op")
                        po[h] = po_t
                    npairs = 2 * tb + 2
                    s_ilv = cfg.get("s_ilv", False)
                    sb_pipe = cfg.get("sb_pipe", False)

                    def emit_pair_S(p):
                        """S mms + exp + mask for pair p; returns pts."""
                        mask_mm_ = cfg.get("mask_mm", False)
                        pts = []
                        for h in range(2):
                            ps = ps_s.tile([128, 1024], F32, tag="sp",
                                           name="ps")
                            for dp in range(2):
                                si = 2 * p + dp
                                r = si - 4 * tb
                                rtrim = (1, 2, 3) if s_bf16 else (1, 2)
                                c0 = 128 * r if r in rtrim else 0
                                diag = mask_mm_ and 0 <= r < 4
                                nc.tensor.matmul(
                                    ps[:, dp * 512 + c0:(dp + 1) * 512],
                                    qT2[64 * h:64 * h + 64,
                                        si * 128:(si + 1) * 128],
                                    kT2[64 * h:64 * h + 64,
                                        tb * 512 + c0:(tb + 1) * 512],
                                    start=True, stop=not diag)
                                if diag:
                                    nc.tensor.matmul(
                                        ps[:, dp * 512 + r * 128:
                                           dp * 512 + (r + 1) * 128],
                                        identb[:], ntri_sb[:],
                                        start=False, stop=True,
                                        skip_group_check=True)
                            pt = ptp.tile([128, 1024], BF16, tag="pt",
                                          name="pt")
                            emit_exp(pt, ps)
                            if not mask_mm_:
                                for dp in range(2):
                                    si = 2 * p + dp
                                    r = si - 4 * tb
                                    if 0 <= r < 4:
                                        sl = pt[:, dp * 512 + r * 128:
                                                dp * 512 + (r + 1) * 128]
                                        nc.vector.tensor_mul(sl, sl,
                                                             mask_sb[:])
                            pts.append(pt)
                        return pts

                    def emit_pair_AV(p, pts):
                        for dp in range(2):
                            si = 2 * p + dp
                            for h in range(2):
                                r = si - 4 * tb
                                c0 = max(r, 0) * 128
                                nc.tensor.matmul(
                                    po[h][:, c0:512],
                                    v2[si][:, h * 65:(h + 1) * 65],
                                    pts[h][:, dp * 512 + c0:
                                           (dp + 1) * 512],
                                    start=(si == 0), stop=(si == slast),
                                    skip_group_check=True)

                    if sb_pipe and not skip_b:
                        # S(p+1) is emitted before AV(p) so the PE queue
                        # always has the next pair's S ready for ACT.
                        if z_defer and st["prev"] is not None:
                            emit_z(*st.pop("prev"))
                            st["prev"] = None
                        prev_pts = emit_pair_S(0)
                        for p in range(1, npairs):
                            pts = emit_pair_S(p)
                            emit_pair_AV(p - 1, prev_pts)
                            prev_pts = pts
                        emit_pair_AV(npairs - 1, prev_pts)
                        npairs = 0  # skip the plain loop below
                    mask_mm = cfg.get("mask_mm", False)
                    for p in range(npairs):
                        pts = []
                        psh = {}
                        if s_ilv:
                            # interleave h0/h1 S mms (rows 0-63 vs 64-127)
                            # so the PE row-tiles them concurrently
                            for h in range(2):
                                psh[h] = ps_s.tile([128, 1024], F32,
                                                   tag="sp", name="psh")
                            for dp in range(2):
                                si = 2 * p + dp
                                r = si - 4 * tb
                                rtrim = (1, 2, 3) if s_bf16 else (1, 2)
                                c0 = 128 * r if r in rtrim else 0
                                for h in range(2):
                                    nc.tensor.matmul(
                                        psh[h][:, dp * 512 + c0:
                                               (dp + 1) * 512],
                                        qT2[64 * h:64 * h + 64,
                                            si * 128:(si + 1) * 128],
                                        kT2[64 * h:64 * h + 64,
                                            tb * 512 + c0:(tb + 1) * 512],
                                        start=True, stop=True)
                        for h in range(2):
                            if s_ilv:
                                ps = psh[h]
                            else:
                                ps = ps_s.tile([128, 1024], F32, tag="sp")
                                for dp in range(2):
                                    si = 2 * p + dp
                                    r = si - 4 * tb
                                    rtrim = (1, 2, 3) if s_bf16 else (1, 2)
                                    c0 = 128 * r if r in rtrim else 0
                                    diag = mask_mm and 0 <= r < 4
                                    nc.tensor.matmul(
                                        ps[:, dp * 512 + c0:(dp + 1) * 512],
                                        qT2[64 * h:64 * h + 64,
                                            si * 128:(si + 1) * 128],
                                        kT2[64 * h:64 * h + 64,
                                            tb * 512 + c0:(tb + 1) * 512],
                                        start=True, stop=not diag)
                                    if diag:
                                        # accumulate -30*strict_lower_tri
                                        # into the diagonal 128-col chunk
                                        nc.tensor.matmul(
                                            ps[:, dp * 512 + r * 128:
                                               dp * 512 + (r + 1) * 128],
                                            identb[:], ntri_sb[:],
                                            start=False, stop=True,
                                            skip_group_check=True)
                            pt = ptp.tile([128, 1024], BF16, tag="pt")
                            if cfg.get("probe_exp_half"):
                                # TIMING PROBE ONLY (wrong numerics): exp
                                # half the tile to test ACT-boundedness
                                nc.scalar.activation(pt[:, 0:512],
                                                     ps[:, 0:512],
                                                     EXP, bias=ebias[:])
                            elif exp_trim and 2 * p >= 4 * tb:
                                # diagonal pair: exp only live cols per half
                                for dp in range(2):
                                    r = 2 * p + dp - 4 * tb
                                    c0 = 128 * r if r in (1, 2, 3) else 0
                                    nc.scalar.activation(
                                        pt[:, dp * 512 + c0:(dp + 1) * 512],
                                        ps[:, dp * 512 + c0:(dp + 1) * 512],
                                        EXP, bias=ebias[:])
                            else:
                                emit_exp(pt, ps)
                            for dp in range(0 if (cfg.get("no_mask")
                                                  or mask_mm) else 2):
                                si = 2 * p + dp
                                r = si - 4 * tb
                                if 0 <= r < 4:
                                    sl = pt[:, dp * 512 + r * 128:
                                            dp * 512 + (r + 1) * 128]
                                    meng = (nc.gpsimd if cfg.get("mask_pool")
                                            else nc.vector)
                                    meng.tensor_mul(sl, sl, mask_sb[:])
                            pts.append(pt)
                        for dp in range(2 if not skip_b else 0):
                            si = 2 * p + dp
                            for h in range(2):
                                r = si - 4 * tb
                                c0 = max(r, 0) * 128
                                nc.tensor.matmul(
                                    po[h][:, c0:512],
                                    v2[si][:, h * 65:(h + 1) * 65],
                                    pts[h][:, dp * 512 + c0:
                                           (dp + 1) * 512],
                                    start=(si == 0), stop=(si == slast),
                                    skip_group_check=True)
                        if z_defer and p == 0 and st["prev"] is not None:
                            emit_z(*st.pop("prev"))
                            st["prev"] = None

                    # ---- normalize (rows 1:65 / row 0) + partial z ----
                    if not skip_b and not skip_z:
                        outT = smallp.tile([128, 512],
                                           BF16 if z_bf16 else F32R,
                                           tag="outT")
                        po_copy = cfg.get("po_copy", False)
                        for h in range(2):
                            if po_copy:
                                # single fast eviction frees the po bank;
                                # normalize then runs off SBUF at leisure
                                posb = smallp.tile([65, 512], F32,
                                                   tag="posb", name="posb")
                                nc.vector.tensor_copy(posb[:], po[h][:])
                                src = posb
                            else:
                                src = po[h]
                            rrow = smallp.tile([1, 512], F32, tag="rrow")
                            nc.vector.reciprocal(rrow[:], src[64:65, :])
                            rbc = smallp.tile([64, 512], F32, tag="rbc")
                            nc.gpsimd.partition_broadcast(rbc[:], rrow[:])
                            nc.vector.tensor_mul(
                                outT[64 * h:64 * h + 64, :],
                                src[0:64, :], rbc[:])
                        if z_defer:
                            st["prev"] = (outT, b, tb)
                        else:
                            emit_z(outT, b, tb)

                if cfg.get("allfront"):
                    # ALL proj/v2 for both batches first (PE-only prologue),
                    # then all 8 attention blocks back-to-back so ACT never
                    # starves; next iteration's prologue overlaps this
                    # iteration's ACT tail through the FIFO queues.
                    st_all = {bb: {"xth": [[None] * 4 for _ in range(NEC)],
                                   "v2": [None] * NSC} for bb in range(B)}
                    cur = [None]

                    def use_b(bb):
                        if cur[0] == bb:
                            return
                        prev = st.get("prev")
                        if cur[0] is not None:
                            st_all[cur[0]] = {k: v for k, v in st.items()
                                              if k != "prev"}
                        st.clear()
                        st.update(st_all[bb])
                        st["prev"] = prev
                        cur[0] = bb

                    for bb in range(B):
                        use_b(bb)
                        for qt in range(4):
                            emit_xth(bb, qt)
                        for nm in ("q", "k", "v"):
                            for tp2 in range(2):
                                emit_proj_chunk(nm, tp2)
                        for s in range(NSC):
                            emit_v2(s)
                    for bb in range(B):
                        use_b(bb)
                        for tb in range(NTB if not skip_attn else 0):
                            emit_stageB(bb, tb)
                elif cfg.get("pipe2"):
                    # explicit cross-batch software pipeline: b=1 proj blocks
                    # are emitted between b=0's heavy attention blocks so the
                    # PE has work while ACT chews on exp.
                    st_all = {bb: {"xth": [[None] * 4 for _ in range(NEC)],
                                   "v2": [None] * NSC} for bb in range(B)}
                    cur = [None]

                    def use_b(bb):
                        if cur[0] == bb:
                            return
                        prev = st.get("prev")
                        if cur[0] is not None:
                            st_all[cur[0]] = {k: v for k, v in st.items()
                                              if k != "prev"}
                        st.clear()
                        st.update(st_all[bb])
                        st["prev"] = prev
                        cur[0] = bb

                    seq = [("x", 0, 0), ("x", 0, 1), ("P", 0, 0),
                           ("V", 0, 0), ("A", 0, 0), ("A", 0, 1),
                           ("x", 0, 2), ("x", 0, 3), ("P", 0, 1),
                           ("V", 0, 1), ("A", 0, 2),
                           ("x", 1, 0), ("x", 1, 1), ("P", 1, 0),
                           ("V", 1, 0), ("A", 0, 3),
                           ("A", 1, 0), ("A", 1, 1),
                           ("x", 1, 2), ("x", 1, 3), ("P", 1, 1),
                           ("V", 1, 1), ("A", 1, 2), ("A", 1, 3)]
                    for op, bb, i in seq:
                        use_b(bb)
                        if op == "x":
                            emit_xth(bb, i)
                        elif op == "P":
                            for nm in ("q", "k", "v"):
                                emit_proj_chunk(nm, i)
                        elif op == "V":
                            for s in range(8 * i, 8 * i + 8):
                                emit_v2(s)
                        else:
                            emit_stageB(bb, i)
                else:
                    for b in range(B):
                        st["xth"] = [[None] * 4 for _ in range(NEC)]
                        st["v2"] = [None] * NSC
                        if interleave:
                            for tp2 in range(2):
                                for qt in (2 * tp2, 2 * tp2 + 1):
                                    emit_xth(b, qt)
                                for nm in ("q", "k", "v"):
                                    emit_proj_chunk(nm, tp2)
                                for s in range(8 * tp2, 8 * tp2 + 8):
                                    emit_v2(s)
                                if not skip_attn:
                                    emit_stageB(b, 2 * tp2)
                                    emit_stageB(b, 2 * tp2 + 1)
                        else:
                            for qt in range(4):
                                emit_xth(b, qt)
                            for nm in ("q", "k", "v"):
                                for tp2 in range(2):
                                    emit_proj_chunk(nm, tp2)
                            for s in range(NSC):
                                emit_v2(s)
                            for tb in range(NTB if not skip_attn else 0):
                                emit_stageB(b, tb)
                if z_defer and st["prev"] is not None:
                    emit_z(*st["prev"])

            if rep == 1:
                body()
            elif cfg.get("unroll"):
                for _ in range(rep):
                    body()
            elif cfg.get("sreset"):
                with tc.For_i(0, rep, 1, staggered_reset=True):
                    body()
            else:
                with tc.For_i(0, rep, 1):
                    body()

    nc.compile()
    return nc


def build_nc_bh(rep=1, cfg=None):
    """Batch x head sharding: core c owns batch c//4 and heads
    4*(c%4)..4*(c%4)+3 (two pairs hp=0,1). All-bf16 matmul path.
    z partial accumulates both pairs in PSUM; host sums 4 cores/batch."""
    cfg = dict(cfg or {})
    sp_bufs = cfg.get("sp_bufs", 2)
    op_bufs = cfg.get("op_bufs", 2)
    pt_bufs = cfg.get("pt_bufs", 4)
    out_q = cfg.get("out_q", "sync")
    exp_trim = cfg.get("exp_trim", True)
    zevict = cfg.get("zevict", "any")  # any|vector|scalar|pool
    NHP = 2                     # head pairs per core
    nc = bacc.Bacc("TRN2", target_bir_lowering=False, debug=False,
                   num_devices=N_CORES)

    xt = nc.dram_tensor("xt", [E, T], BF16, kind="ExternalInput").ap()
    wq = nc.dram_tensor("wq", [E, NHP * F], BF16, kind="ExternalInput").ap()
    wk = nc.dram_tensor("wk", [E, NHP * F], BF16, kind="ExternalInput").ap()
    wv = nc.dram_tensor("wv", [E, NHP * F], BF16, kind="ExternalInput").ap()
    wot = nc.dram_tensor("wot", [NHP * F, E], BF16,
                         kind="ExternalInput").ap()
    mask = nc.dram_tensor("mask", [128, 128], BF16, kind="ExternalInput").ap()
    zp = nc.dram_tensor("zp", [T, E], F16, kind="ExternalOutput").ap()

    with tile.TileContext(nc) as tc:
        with (
            tc.tile_pool(name="const", bufs=1) as cpool,
            tc.tile_pool(name="xtp", bufs=36) as xtp,
            tc.tile_pool(name="proj", bufs=2) as projp,
            tc.tile_pool(name="v2p", bufs=2 * NSC + 8) as v2p,
            tc.tile_pool(name="ptp", bufs=pt_bufs) as ptp,
            tc.tile_pool(name="outTp",
                         bufs=(5 if cfg.get("z_defer") else 3)) as outTp,
            tc.tile_pool(name="smallp", bufs=4) as smallp,
            tc.tile_pool(name="zsbp", bufs=3) as zsbp,
            tc.tile_pool(name="ps_s", bufs=sp_bufs, space="PSUM") as ps_s,
            tc.tile_pool(name="ps_o", bufs=op_bufs, space="PSUM") as ps_o,
            tc.tile_pool(name="ps_t", bufs=2, space="PSUM") as ps_t,
        ):
            # ---- constants (loaded once) ----
            ident = cpool.tile([128, 128], F32, tag="ident")
            make_identity(nc, ident[:])
            mask_sb = cpool.tile([128, 128], BF16, tag="mask")
            nc.scalar.dma_start(mask_sb[:], mask)
            ebias = cpool.tile([128, 1], F32, tag="ebias")
            nc.vector.memset(ebias[:], EXP_BIAS)
            # weights: [E, 2F] -> [128, NEC*2F]; w[hp][e] = [128, F]
            wsb = {}
            for wsrc, nm in ((wq, "wq"), (wk, "wk"), (wv, "wv")):
                t_ = cpool.tile([128, NEC * NHP * F], BF16, tag=f"wall{nm}")
                nc.scalar.dma_start(
                    t_.rearrange("p (a c) -> p a c", a=NEC),
                    wsrc.rearrange("(a p) c -> p a c", p=128))
                wsb[nm] = [[t_[:, e * NHP * F + hp * F:
                               e * NHP * F + (hp + 1) * F]
                            for e in range(NEC)] for hp in range(NHP)]
            wot_sb = []
            for hp in range(NHP):
                t_ = cpool.tile([F, E], BF16, tag=f"wot{hp}")
                nc.scalar.dma_start(t_[:], wot[hp * F:(hp + 1) * F, :])
                wot_sb.append(t_)

            def body():
                xth = [[None] * 4 for _ in range(NEC)]
                qT2, kT2, vT2, v2 = {}, {}, {}, {hp: [None] * NSC
                                                 for hp in range(NHP)}
                z_defer = cfg.get("z_defer", False)
                interleave = cfg.get("interleave", False)

                def emit_xth(qt):
                    for e in range(NEC):
                        t_ = xtp.tile([128, T // 4], BF16, tag="xt")
                        eng = ((nc.sync, nc.scalar)[e % 2]
                               if cfg.get("xt_q") else nc.sync)
                        eng.dma_start(
                            t_[:], xt[e * 128:(e + 1) * 128,
                                      qt * (T // 4):(qt + 1) * (T // 4)])
                        xth[e][qt] = t_

                def emit_proj_chunk(hp, nm, tp2):
                    if tp2 == 0:
                        pdt = F32 if nm == "v" else BF16
                        dst = projp.tile([128, T], pdt, tag=f"{nm}T2_{hp}")
                        {"q": qT2, "k": kT2, "v": vT2}[nm][hp] = dst
                    dst = {"q": qT2, "k": kT2, "v": vT2}[nm][hp]
                    ps = ps_s.tile([128, 1024], F32, tag="sp")
                    for half in range(2):
                        c0 = tp2 * 1024 + half * 512
                        for e in range(NEC):
                            nc.tensor.matmul(
                                ps[:, half * 512:(half + 1) * 512],
                                wsb["w" + nm][hp][e],
                                xth[e][c0 // 512][:],
                                start=(e == 0), stop=(e == NEC - 1))
                    nc.any.tensor_copy(
                        dst[:, tp2 * 1024:(tp2 + 1) * 1024], ps[:])

                def emit_v2(hp, s):
                    tpw = ps_t.tile([128, 512], F32, tag="tp")
                    tp_ = tpw[:, 0:128]
                    nc.tensor.matmul(tp_[:],
                                     vT2[hp][:, s * 128:(s + 1) * 128],
                                     ident[:], is_transpose=True)
                    v2t = v2p.tile([128, 130], BF16, tag="v2")
                    v2r = v2t.rearrange("p (g c) -> p g c", g=2)
                    nc.vector.memset(v2r[:, :, 64:65], 1.0)
                    nc.vector.tensor_copy(
                        v2r[:, :, 0:64],
                        tp_.rearrange("p (g c) -> p g c", g=2))
                    v2[hp][s] = v2t

                def emit_z(zoutT, ztb):
                    for jp2 in range(2):
                        zsb = zsbp.tile([128, 2048], F16, tag="zsb")
                        for jj in range(2):
                            j = 2 * jp2 + jj
                            zpsl = []
                            for eb in range(2):
                                zps = ps_t.tile([128, 512], F32, tag="tp")
                                zpsl.append(zps)
                            for hp in range(NHP):
                                for eb in range(2):
                                    nc.tensor.matmul(
                                        zpsl[eb][:],
                                        zoutT[hp][:, j * 128:(j + 1) * 128],
                                        wot_sb[hp][:, eb * 512:(eb + 1) * 512],
                                        start=(hp == 0), stop=(hp == NHP - 1))
                            for eb in range(2):
                                dstsl = zsb[:, jj * 1024 + eb * 512:
                                            jj * 1024 + (eb + 1) * 512]
                                if zevict == "vector":
                                    nc.vector.tensor_copy(dstsl, zpsl[eb][:])
                                elif zevict == "scalar":
                                    nc.scalar.copy(dstsl, zpsl[eb][:])
                                else:
                                    nc.any.tensor_copy(dstsl, zpsl[eb][:])
                        t0r = (4 * ztb + 2 * jp2) * 128
                        getattr(nc, out_q).dma_start(
                            zp[t0r:t0r + 256, :]
                            .rearrange("(a p) c -> p a c", p=128),
                            zsb.rearrange("p (a c) -> p a c", a=2))

                prev_outT = None

                def emit_stageB(tb):
                    nonlocal prev_outT
                    slast = 4 * tb + 3
                    outT = {}
                    for hp in range(NHP):
                        po = {}
                        for h in range(2):
                            po_t = ps_o.tile([65, 512], F32, tag="op")
                            po[h] = po_t
                        npairs = 2 * tb + 2
                        for p in range(npairs):
                            pts = []
                            for h in range(2):
                                ps = ps_s.tile([128, 1024], F32, tag="sp")
                                for dp in range(2):
                                    si = 2 * p + dp
                                    r = si - 4 * tb
                                    c0 = 128 * r if r in (1, 2, 3) else 0
                                    nc.tensor.matmul(
                                        ps[:, dp * 512 + c0:(dp + 1) * 512],
                                        qT2[hp][64 * h:64 * h + 64,
                                                si * 128:(si + 1) * 128],
                                        kT2[hp][64 * h:64 * h + 64,
                                                tb * 512 + c0:(tb + 1) * 512],
                                        start=True, stop=True)
                                pt = ptp.tile([128, 1024], BF16, tag="pt")
                                if exp_trim and 2 * p >= 4 * tb:
                                    # diagonal pair: exp live cols per half
                                    for dp in range(2):
                                        r = 2 * p + dp - 4 * tb
                                        c0 = 128 * r if r in (1, 2, 3) else 0
                                        nc.scalar.activation(
                                            pt[:, dp * 512 + c0:
                                               (dp + 1) * 512],
                                            ps[:, dp * 512 + c0:
                                               (dp + 1) * 512],
                                            EXP, bias=ebias[:])
                                else:
                                    nc.scalar.activation(pt[:], ps[:], EXP,
                                                         bias=ebias[:])
                                for dp in range(2):
                                    si = 2 * p + dp
                                    r = si - 4 * tb
                                    if 0 <= r < 4:
                                        sl = pt[:, dp * 512 + r * 128:
                                                dp * 512 + (r + 1) * 128]
                                        nc.vector.tensor_mul(sl, sl,
                                                             mask_sb[:])
                                pts.append(pt)
                            for dp in range(2):
                                si = 2 * p + dp
                                for h in range(2):
                                    r = si - 4 * tb
                                    c0 = max(r, 0) * 128
                                    nc.tensor.matmul(
                                        po[h][:, c0:512],
                                        v2[hp][si][:, h * 65:(h + 1) * 65],
                                        pts[h][:, dp * 512 + c0:
                                               (dp + 1) * 512],
                                        start=(si == 0), stop=(si == slast),
                                        skip_group_check=True)
                            # deferred z of the previous t-block slots into
                            # the PE queue here, after deps are long ready
                            if z_defer and hp == 0 and p == 0 and tb > 0:
                                emit_z(prev_outT, tb - 1)

                        # ---- normalize (rows 0:64 / row 64) ----
                        oT = outTp.tile([128, 512], BF16, tag="outT")
                        for h in range(2):
                            rrow = smallp.tile([1, 512], F32, tag="rrow")
                            nc.vector.reciprocal(rrow[:], po[h][64:65, :])
                            rbc = smallp.tile([64, 512], F32, tag="rbc")
                            nc.gpsimd.partition_broadcast(rbc[:], rrow[:])
                            nc.vector.tensor_mul(
                                oT[64 * h:64 * h + 64, :],
                                po[h][0:64, :], rbc[:])
                        outT[hp] = oT

                    # ---- z: accumulate both pairs in PSUM ----
                    if z_defer:
                        prev_outT = outT
                    else:
                        emit_z(outT, tb)

                if interleave:
                    for tp2 in range(2):
                        for qt in (2 * tp2, 2 * tp2 + 1):
                            emit_xth(qt)
                        for hp in range(NHP):
                            for nm in ("q", "k", "v"):
                                emit_proj_chunk(hp, nm, tp2)
                        for hp in range(NHP):
                            for s in range(8 * tp2, 8 * tp2 + 8):
                                emit_v2(hp, s)
                        emit_stageB(2 * tp2)
                        emit_stageB(2 * tp2 + 1)
                else:
                    for qt in range(4):
                        emit_xth(qt)
                    for hp in range(NHP):
                        for nm in ("q", "k", "v"):
                            for tp2 in range(2):
                                emit_proj_chunk(hp, nm, tp2)
                        for s in range(NSC):
                            emit_v2(hp, s)
                    for tb in range(NTB):
                        emit_stageB(tb)
                if z_defer and prev_outT is not None:
                    emit_z(prev_outT, NTB - 1)

            if rep == 1:
                body()
            elif cfg.get("unroll"):
                for _ in range(rep):
                    body()
            elif cfg.get("body2") and (rep - 1) % 2 == 0:
                with tc.For_i(0, (rep - 1) // 2, 1):
                    body()
                    body()
                body()
            elif cfg.get("sreset"):
                with tc.For_i(0, rep, 1, staggered_reset=True):
                    body()
            else:
                with tc.For_i(0, rep, 1):
                    body()

    nc.compile()
    return nc


def make_in_maps_bh(inputs, Wk, Wq, Wv, Wo):
    """Shard: core c gets batch c//4, heads 4*(c%4)..4*(c%4)+3."""
    bf = ml_dtypes.bfloat16
    scale = np.float32(D ** -0.5)
    tri = (np.arange(128)[None, :] >= np.arange(128)[:, None])
    mask = tri.astype(bf)
    in_maps = []
    for c in range(N_CORES):
        b = c // 4
        h0 = 4 * (c % 4)
        xt = np.ascontiguousarray(inputs[b].T).astype(bf)
        wq2 = np.concatenate([Wq[h0 + i] for i in range(4)], axis=1)
        wk2 = np.concatenate([Wk[h0 + i] for i in range(4)], axis=1) * scale
        wv2 = np.concatenate([Wv[h0 + i] for i in range(4)], axis=1)
        wot = np.ascontiguousarray(Wo[:, 64 * h0:64 * (h0 + 4)].T)
        in_maps.append({
            "xt": xt,
            "wq": np.ascontiguousarray(wq2).astype(bf),
            "wk": np.ascontiguousarray(wk2).astype(bf),
            "wv": np.ascontiguousarray(wv2).astype(bf),
            "wot": wot.astype(bf),
            "mask": mask,
        })
    return in_maps


def make_in_maps(inputs, Wk, Wq, Wv, Wo, xt_bf16=False, z_bf16=False):
    """Shard full inputs into per-core input maps."""
    wdt = ml_dtypes.bfloat16 if xt_bf16 else np.float32
    zdt = ml_dtypes.bfloat16 if z_bf16 else np.float32
    xt = np.ascontiguousarray(inputs.transpose(0, 2, 1)).astype(wdt)
    scale = np.float32(D ** -0.5)
    tri = (np.arange(128)[None, :] >= np.arange(128)[:, None])
    mask = tri.astype(ml_dtypes.bfloat16)
    # -30 on strictly-lower (k > j): masks S[s,t] where s > t via matmul
    ntri = (-30.0 * (np.arange(128)[:, None] > np.arange(128)[None, :])
            ).astype(ml_dtypes.bfloat16)
    in_maps = []
    for c in range(N_CORES):
        h0 = HPC * c
        wq2 = np.ascontiguousarray(
            np.concatenate([Wq[h0 + i] for i in range(HPC)], axis=1))
        wk2 = np.ascontiguousarray(
            np.concatenate([Wk[h0 + i] for i in range(HPC)], axis=1)) * scale
        wv2 = np.ascontiguousarray(
            np.concatenate([Wv[h0 + i] for i in range(HPC)], axis=1))
        wot = np.ascontiguousarray(Wo[:, F * c:F * (c + 1)].T)
        in_maps.append({
            "xt": xt,
            "wq": wq2.astype(wdt),
            "wk": wk2.astype(wdt),
            "wv": wv2.astype(wdt),
            "wot": wot.astype(zdt),
            "mask": mask,
            "ntri": ntri,
        })
    return in_maps


_NC = None
_RUN = None
DEFAULT_CFG = {"any_copy": True, "out_q": "sync", "xt_bf16": True,
               "s_bf16": True, "z_bf16": True,
               "interleave": True, "z_defer": True, "sreset": True}


def combine(zp_list, bo, cfg):
    """Combine per-core zp partials into the full [B, T, E] output."""
    z = np.zeros((B, T, E), dtype=np.float32)
    if cfg.get("bh"):
        for c in range(N_CORES):
            z[c // 4] += np.asarray(zp_list[c]).astype(np.float32)
    else:
        for c in range(N_CORES):
            z += np.asarray(zp_list[c]).astype(np.float32)
    return z + bo.astype(np.float32)


def kernel(inputs, Wk, Wq, Wv, Wo, bo):
    global _NC, _RUN
    if _NC is None:
        _NC = build_nc(cfg=DEFAULT_CFG)
    if DEFAULT_CFG.get("bh"):
        in_maps = make_in_maps_bh(inputs, Wk, Wq, Wv, Wo)
    else:
        in_maps = make_in_maps(inputs, Wk, Wq, Wv, Wo,
                               xt_bf16=DEFAULT_CFG.get("xt_bf16", False),
                               z_bf16=DEFAULT_CFG.get("z_bf16", False))
    # drop inputs the built kernel does not declare (e.g. ntri w/o mask_mm)
    declared = {
        a.memorylocations[0].name
        for a in _NC.m.functions[0].allocations
        if isinstance(a, mybir.MemoryLocationSet) and a.kind == "ExternalInput"
    }
    in_maps = [{k: v for k, v in m.items() if k in declared}
               for m in in_maps]
    try:
        if _RUN is None:
            _RUN = _make_runner(_NC)
        results = _RUN(in_maps)
    except Exception:
        _RUN = False if _RUN is None else _RUN
        res = run_bass_kernel_spmd(_NC, in_maps,
                                   core_ids=list(range(N_CORES)))
        results = res.results
    return combine([results[c]["zp"] for c in range(N_CORES)], bo,
                   DEFAULT_CFG)



# revision 31
# speedup vs baseline: 1.1909x; 1.0584x over previous
"""Multi-head causal attention (B=2, T=2048, E=1024, H=16, D=64) on 8 TRN2 cores.

Sharding: tensor-parallel over heads. Core c owns heads {2c, 2c+1} for both
batches. Each core computes its heads' q/k/v projections, causal attention,
and a partial output projection z_c = out_c @ Wo[:, 128c:128c+128].T.
Host combines: z = sum_c z_c + bo.

Note the reference computes wei = K @ Q^T, i.e. output token t attends over
s <= t with logits k_t . q_s. We compute ST[s, t] = q_s . k_t (s on
partitions) so that the A@V matmul needs no transposes, and get the softmax
denominator via a ones-column appended to V.
"""

import numpy as np
import ml_dtypes

import concourse.bacc as bacc
import concourse.mybir as mybir
import concourse.tile as tile
from concourse.bass_utils import run_bass_kernel_spmd
from concourse.masks import make_identity


def _make_runner(nc):
    """Persistent jitted SPMD callable (avoids per-call jit re-trace)."""
    import jax
    from jax.sharding import Mesh, NamedSharding, PartitionSpec
    try:
        from jax.experimental.shard_map import shard_map
    except ImportError:
        shard_map = jax.shard_map
    from concourse.bass2jax import (_bass_exec_p, install_neuronx_cc_hook,
                                    partition_id_tensor)

    install_neuronx_cc_hook()
    partition_name = (nc.partition_id_tensor.name
                      if nc.partition_id_tensor else None)
    in_names, out_names, out_avals, zero_outs = [], [], [], []
    for alloc in nc.m.functions[0].allocations:
        if not isinstance(alloc, mybir.MemoryLocationSet):
            continue
        name = alloc.memorylocations[0].name
        if alloc.kind == "ExternalInput":
            if name != partition_name:
                in_names.append(name)
        elif alloc.kind == "ExternalOutput":
            shape = tuple(alloc.tensor_shape)
            dtype = mybir.dt.np(alloc.dtype)
            out_names.append(name)
            out_avals.append(jax.core.ShapedArray(shape, dtype))
            zero_outs.append(np.zeros(shape, dtype))
    n_params = len(in_names)
    all_in = list(in_names) + list(out_names)
    if partition_name is not None:
        all_in.append(partition_name)

    def _body(*args):
        operands = list(args)
        if partition_name is not None:
            operands.append(partition_id_tensor())
        return tuple(_bass_exec_p.bind(
            *operands, out_avals=tuple(out_avals), in_names=tuple(all_in),
            out_names=tuple(out_names), lowering_input_output_aliases=(),
            sim_require_finite=True, sim_require_nnan=True, nc=nc))

    devices = jax.devices()[:N_CORES]
    mesh = Mesh(np.asarray(devices), ("core",))
    spec = NamedSharding(mesh, PartitionSpec("core"))
    rspec = NamedSharding(mesh, PartitionSpec())
    # inputs identical on every core are sent once and replicated
    replicated = {"xt", "mask", "ntri"}
    in_specs = tuple(
        (PartitionSpec() if nm in replicated else PartitionSpec("core"))
        for nm in in_names) + (PartitionSpec("core"),) * len(out_names)
    fn = jax.jit(
        shard_map(_body, mesh=mesh, in_specs=in_specs,
                  out_specs=(PartitionSpec("core"),) * len(out_names),
                  check_rep=False),
        keep_unused=True)
    zeros_dev = [
        jax.device_put(np.zeros((N_CORES * z.shape[0], *z.shape[1:]), z.dtype),
                       spec) for z in zero_outs
    ]

    def run(in_maps):
        concat = [
            jax.device_put(np.asarray(in_maps[0][nm]), rspec)
            if nm in replicated else
            jax.device_put(
                np.concatenate([np.asarray(in_maps[c][nm])
                                for c in range(N_CORES)], axis=0), spec)
            for nm in in_names
        ]
        outs = fn(*concat, *zeros_dev)
        fulls = [np.asarray(outs[i]).reshape(N_CORES, *out_avals[i].shape)
                 for i in range(len(out_names))]
        return [{nm: fulls[i][c] for i, nm in enumerate(out_names)}
                for c in range(N_CORES)]

    return run

N_CORES = 8
B, T, E = 2, 2048, 1024
H, D = 16, 64
HPC = H // N_CORES          # heads per core = 2
F = HPC * D                 # local feature cols = 128
TBLK = 512                  # t-block width for stage A
NTB = T // TBLK             # 4
NSC = T // 128              # s-chunks = 16
NEC = E // 128              # e-chunks = 8
EXP_BIAS = -2.0             # exp(S + EXP_BIAS); cancels in softmax, guards overflow

F32 = mybir.dt.float32
F16 = mybir.dt.float16
F32R = mybir.dt.float32r
BF16 = mybir.dt.bfloat16
EXP = mybir.ActivationFunctionType.Exp


def build_nc(rep=1, cfg=None):
    cfg = dict(cfg or {})
    if cfg.get("bh"):
        return build_nc_bh(rep, cfg)
    any_copy = cfg.get("any_copy", False)
    evict = cfg.get("evict", "mixed")  # mixed|zscalar|zvector
    sp_bufs = cfg.get("sp_bufs", 2)
    op_bufs = cfg.get("op_bufs", 2)
    misc_bufs = cfg.get("misc_bufs", None)  # if set, tp+zp merged [128,512] x misc_bufs
    pt_bufs = cfg.get("pt_bufs", 4)
    out_q = cfg.get("out_q", "scalar")  # engine for output DMAs
    xt_bf16 = cfg.get("xt_bf16", False)
    s_bf16 = cfg.get("s_bf16", False)
    z_bf16 = cfg.get("z_bf16", False)
    skip_z = cfg.get("skip_z", False)
    skip_b = cfg.get("skip_b", False)
    skip_attn = cfg.get("skip_attn", False)
    td_form = cfg.get("td_form", False)
    sp_wide = cfg.get("sp_wide", False)
    nc = bacc.Bacc("TRN2", target_bir_lowering=False, debug=False,
                   num_devices=N_CORES)

    xt = nc.dram_tensor("xt", [B, E, T], BF16 if xt_bf16 else F32R,
                        kind="ExternalInput").ap()
    wq = nc.dram_tensor("wq", [E, F], BF16 if xt_bf16 else F32R, kind="ExternalInput").ap()
    wk = nc.dram_tensor("wk", [E, F], BF16 if xt_bf16 else F32R, kind="ExternalInput").ap()
    wv = nc.dram_tensor("wv", [E, F], BF16 if xt_bf16 else F32R, kind="ExternalInput").ap()
    wot = nc.dram_tensor("wot", [F, E], BF16 if z_bf16 else F32R,
                         kind="ExternalInput").ap()
    mask = nc.dram_tensor("mask", [128, 128], BF16, kind="ExternalInput").ap()
    if cfg.get("mask_mm"):
        ntri = nc.dram_tensor("ntri", [128, 128], BF16,
                              kind="ExternalInput").ap()
    zp = nc.dram_tensor("zp", [B, T, E], F16, kind="ExternalOutput").ap()

    with tile.TileContext(nc) as tc:
        with (
            tc.tile_pool(name="const", bufs=1) as cpool,
            tc.tile_pool(name="xtp", bufs=cfg.get("xtp_bufs", 36)) as xtp,
            tc.tile_pool(name="proj", bufs=2) as projp,
            tc.tile_pool(name="v2p", bufs=2 * NSC) as v2p,
            tc.tile_pool(name="ptp", bufs=pt_bufs) as ptp,
            tc.tile_pool(name="smallp", bufs=4) as smallp,
            tc.tile_pool(name="zsbp", bufs=3) as zsbp,
            tc.tile_pool(name="ps_s", bufs=sp_bufs, space="PSUM") as ps_s,
            tc.tile_pool(name="ps_o", bufs=op_bufs, space="PSUM") as ps_o,
            tc.tile_pool(name="ps_t", bufs=(misc_bufs or 2), space="PSUM") as ps_t,
        ):
            # ---- constants (loaded once) ----
            v_bf16 = cfg.get("v_bf16", False)
            ident = cpool.tile([128, 128], BF16 if v_bf16 else F32,
                               tag="ident")
            make_identity(nc, ident[:])
            mask_sb = cpool.tile([128, 128], BF16, tag="mask")
            nc.scalar.dma_start(mask_sb[:], mask)
            ebias = cpool.tile([128, 1], F32, tag="ebias")
            nc.vector.memset(ebias[:], EXP_BIAS)
            if cfg.get("mask_mm"):
                # -30 * strict-lower-tri constant and bf16 identity: the
                # causal mask is accumulated into S by one extra matmul
                # (ident.T @ ntri) instead of a DVE multiply after exp.
                ntri_sb = cpool.tile([128, 128], BF16, tag="ntri")
                nc.scalar.dma_start(ntri_sb[:], ntri)
                identb = cpool.tile([128, 128], BF16, tag="identb")
                make_identity(nc, identb[:])
            # one coalesced DMA per weight tensor: [E, F] -> [128, NEC*F]
            wq_sb = []
            wk_sb = []
            wv_sb = []
            for lst, wsrc, nm in ((wq_sb, wq, "wq"), (wk_sb, wk, "wk"),
                                  (wv_sb, wv, "wv")):
                t_ = cpool.tile([128, NEC * F], BF16 if xt_bf16 else F32R,
                                tag=f"wall{nm}")
                nc.scalar.dma_start(
                    t_.rearrange("p (a c) -> p a c", a=NEC),
                    wsrc.rearrange("(a p) c -> p a c", p=128))
                for e in range(NEC):
                    lst.append(t_[:, e * F:(e + 1) * F])
            wot_sb = cpool.tile([F, E], BF16 if z_bf16 else F32R, tag="wot")
            nc.scalar.dma_start(wot_sb[:], wot)

            def body():
                z_defer = cfg.get("z_defer", False)
                interleave = cfg.get("interleave", False)
                st = {"prev": None}

                def emit_xth(b, qt):
                    if cfg.get("xt_wide"):
                        # one [128, T] DMA per e-chunk (4KB/partition line),
                        # issued at qt==0; xth[e][qt] slices the wide tile
                        if qt != 0:
                            return
                        for e in range(NEC):
                            t_ = xtp.tile([128, T], BF16 if xt_bf16 else F32R,
                                          tag="xt", bufs=cfg.get(
                                              "xtw_bufs", 9))
                            eng = ((nc.sync, nc.gpsimd)[e % 2]
                                   if cfg.get("xt_q") else nc.sync)
                            eng.dma_start(
                                t_[:], xt[b, e * 128:(e + 1) * 128, :])
                            for q4 in range(4):
                                st["xth"][e][q4] = t_[:, q4 * (T // 4):
                                                      (q4 + 1) * (T // 4)]
                        return
                    for e in range(NEC):
                        t_ = xtp.tile([128, T // 4],
                                      BF16 if xt_bf16 else F32R, tag="xt")
                        eng = ((nc.sync, nc.gpsimd)[e % 2]
                               if cfg.get("xt_q") else nc.sync)
                        eng.dma_start(
                            t_[:], xt[b, e * 128:(e + 1) * 128,
                                      qt * (T // 4):(qt + 1) * (T // 4)])
                        st["xth"][e][qt] = t_

                def emit_proj_chunk(nm, tp2):
                    wsb = {"q": wq_sb, "k": wk_sb, "v": wv_sb}[nm]
                    if tp2 == 0:
                        if nm == "v":
                            pdt = BF16 if v_bf16 else F32
                        else:
                            pdt = BF16 if s_bf16 else F32R
                        dstn = projp.tile([128, T], pdt, tag=f"{nm}T2")
                        st[nm] = dstn
                    dst = st[nm]
                    if cfg.get("proj_tp"):
                        # proj accumulates in the tp pool (1-bank halves) so
                        # the sp pool stays dedicated to the S/exp pipeline
                        for half in range(2):
                            c0 = tp2 * 1024 + half * 512
                            ph = ps_t.tile([128, 512], F32, tag="tp",
                                           name="ph")
                            for e in range(NEC):
                                nc.tensor.matmul(
                                    ph[:], wsb[e],
                                    st["xth"][e][c0 // 512][:],
                                    start=(e == 0), stop=(e == NEC - 1))
                            (nc.any if any_copy else nc.vector).tensor_copy(
                                dst[:, c0:c0 + 512], ph[:])
                        return
                    ps = ps_s.tile([128, 1024], F32, tag="sp")
                    for half in range(2):
                        c0 = tp2 * 1024 + half * 512
                        for e in range(NEC):
                            nc.tensor.matmul(
                                ps[:, half * 512:(half + 1) * 512],
                                wsb[e],
                                st["xth"][e][c0 // 512][:],
                                start=(e == 0), stop=(e == NEC - 1))
                    (nc.any if any_copy else nc.vector).tensor_copy(
                        dst[:, tp2 * 1024:(tp2 + 1) * 1024], ps[:])

                def emit_v2(s):
                    if v_bf16:
                        tpw = ps_t.tile([128, 512], BF16, tag="tp")
                    else:
                        tpw = ps_t.tile([128, 512], F32, tag="tp")
                    tp_ = tpw[:, 0:128]
                    nc.tensor.matmul(tp_[:], st["v"][:, s * 128:(s + 1) * 128],
                                     ident[:], is_transpose=True)
                    v2t = v2p.tile([128, 130], BF16, tag="v2")
                    v2r = v2t.rearrange("p (g c) -> p g c", g=2)
                    nc.vector.memset(v2r[:, :, 64:65], 1.0)
                    nc.vector.tensor_copy(
                        v2r[:, :, 0:64],
                        tp_.rearrange("p (g c) -> p g c", g=2))
                    st["v2"][s] = v2t

                def emit_z(outT, b, tb):
                    for jp2 in range(2):
                        zsb = zsbp.tile([128, 2048], F16, tag="zsb")
                        for jj in range(2):
                            j = 2 * jp2 + jj
                            for eb in range(2):
                                zps = ps_t.tile([128, 512], F32,
                                                tag="tp")
                                nc.tensor.matmul(
                                    zps[:],
                                    outT[:, j * 128:(j + 1) * 128],
                                    wot_sb[:, eb * 512:(eb + 1) * 512],
                                    start=True, stop=True)
                                dstsl = zsb[:, jj * 1024 + eb * 512:
                                            jj * 1024 + (eb + 1) * 512]
                                if evict == "zscalar":
                                    nc.scalar.copy(dstsl, zps[:])
                                elif evict == "zvector":
                                    nc.vector.tensor_copy(dstsl, zps[:])
                                elif any_copy:
                                    nc.any.tensor_copy(dstsl, zps[:])
                                elif eb == 0:
                                    nc.vector.tensor_copy(dstsl, zps[:])
                                else:
                                    nc.scalar.copy(dstsl, zps[:])
                        t0r = (4 * tb + 2 * jp2) * 128
                        getattr(nc, out_q).dma_start(
                            zp[b, t0r:t0r + 256, :]
                            .rearrange("(a p) c -> p a c", p=128),
                            zsb.rearrange("p (a c) -> p a c", a=2))

                expctr = [0]

                def emit_exp(pt, ps):
                    """exp(ps + EXP_BIAS) -> pt; every k-th tile via DVE
                    Schraudolph approximation (cfg dve_exp = k)."""
                    k = cfg.get("dve_exp", 0)
                    expctr[0] += 1
                    if k and (expctr[0] % k == 0):
                        A = float(2.0 ** 23 / np.log(2.0))
                        Bc = 1065353216.0 - 366393.0 + A * EXP_BIAS
                        ti = ptp.tile([128, 1024], mybir.dt.int32,
                                      tag="ti", bufs=2, name="ti")
                        nc.vector.tensor_scalar(
                            ti[:], ps[:], A, Bc,
                            mybir.AluOpType.mult, mybir.AluOpType.add)
                        nc.vector.tensor_copy(pt[:], ti[:].bitcast(F32))
                    else:
                        nc.scalar.activation(pt[:], ps[:], EXP,
                                             bias=ebias[:])

                def emit_stageB_quad(b, tb):
                    """Quad variant: S in [128,2048] bf16 PSUM (2 banks), one
                    exp per quad, h0/h1 S mms interleaved for row-packing."""
                    qT2, kT2, v2 = st["q"], st["k"], st["v2"]
                    slast = 4 * tb + 3
                    po = {h: ps_o.tile([65, 512], F32, tag="op", name="po")
                          for h in range(2)}
                    nquads = tb + 1
                    for q in range(nquads):
                        psq = {h: ps_s.tile([128, 2048], BF16, tag="sp", name="psq")
                               for h in range(2)}
                        for dp in range(4):
                            si = 4 * q + dp
                            r = si - 4 * tb
                            c0 = 128 * r if r in (1, 2, 3) else 0
                            for h in range(2):
                                nc.tensor.matmul(
                                    psq[h][:, dp * 512 + c0:(dp + 1) * 512],
                                    qT2[64 * h:64 * h + 64,
                                        si * 128:(si + 1) * 128],
                                    kT2[64 * h:64 * h + 64,
                                        tb * 512 + c0:(tb + 1) * 512],
                                    start=True, stop=True)
                        pts = []
                        for h in range(2):
                            pt = ptp.tile([128, 2048], BF16, tag="pt")
                            nc.scalar.activation(pt[:], psq[h][:], EXP,
                                                 bias=ebias[:])
                            for dp in range(4):
                                r = 4 * q + dp - 4 * tb
                                if 0 <= r < 4:
                                    sl = pt[:, dp * 512 + r * 128:
                                            dp * 512 + (r + 1) * 128]
                                    meng = (nc.gpsimd if cfg.get("mask_pool")
                                            else nc.vector)
                                    meng.tensor_mul(sl, sl, mask_sb[:])
                            pts.append(pt)
                        for dp in range(4):
                            si = 4 * q + dp
                            r = si - 4 * tb
                            c0 = max(r, 0) * 128
                            for h in range(2):
                                nc.tensor.matmul(
                                    po[h][:, c0:512],
                                    v2[si][:, h * 65:(h + 1) * 65],
                                    pts[h][:, dp * 512 + c0:(dp + 1) * 512],
                                    start=(si == 0), stop=(si == slast),
                                    skip_group_check=True)
                        if (cfg.get("z_defer") and q == 0
                                and st["prev"] is not None):
                            emit_z(*st.pop("prev"))
                            st["prev"] = None

                    # ---- normalize (rows 1:65 / row 0) + partial z ----
                    if not skip_z:
                        outT = smallp.tile([128, 512],
                                           BF16 if z_bf16 else F32R,
                                           tag="outT")
                        for h in range(2):
                            rrow = smallp.tile([1, 512], F32, tag="rrow")
                            nc.vector.reciprocal(rrow[:], po[h][64:65, :])
                            rbc = smallp.tile([64, 512], F32, tag="rbc")
                            nc.gpsimd.partition_broadcast(rbc[:], rrow[:])
                            nc.vector.tensor_mul(
                                outT[64 * h:64 * h + 64, :],
                                po[h][0:64, :], rbc[:])
                        if cfg.get("z_defer"):
                            st["prev"] = (outT, b, tb)
                        else:
                            emit_z(outT, b, tb)

                def emit_stageB(b, tb):
                    if cfg.get("quad"):
                        return emit_stageB_quad(b, tb)
                    qT2, kT2, vT2, v2 = st["q"], st["k"], st["v"], st["v2"]
                    exp_trim = cfg.get("exp_trim", False)
                    slast = 4 * tb + 3
                    po = {}
                    for h in range(2 if not skip_b else 0):
                        po_t = ps_o.tile([65, 512], F32, tag="op")
                        po[h] = po_t
                    npairs = 2 * tb + 2
                    s_ilv = cfg.get("s_ilv", False)
                    sb_pipe = cfg.get("sb_pipe", False)

                    def emit_pair_S(p):
                        """S mms + exp + mask for pair p; returns pts."""
                        mask_mm_ = cfg.get("mask_mm", False)
                        pts = []
                        for h in range(2):
                            ps = ps_s.tile([128, 1024], F32, tag="sp",
                                           name="ps")
                            for dp in range(2):
                                si = 2 * p + dp
                                r = si - 4 * tb
                                rtrim = (1, 2, 3) if s_bf16 else (1, 2)
                                c0 = 128 * r if r in rtrim else 0
                                diag = mask_mm_ and 0 <= r < 4
                                nc.tensor.matmul(
                                    ps[:, dp * 512 + c0:(dp + 1) * 512],
                                    qT2[64 * h:64 * h + 64,
                                        si * 128:(si + 1) * 128],
                                    kT2[64 * h:64 * h + 64,
                                        tb * 512 + c0:(tb + 1) * 512],
                                    start=True, stop=not diag)
                                if diag:
                                    nc.tensor.matmul(
                                        ps[:, dp * 512 + r * 128:
                                           dp * 512 + (r + 1) * 128],
                                        identb[:], ntri_sb[:],
                                        start=False, stop=True,
                                        skip_group_check=True)
                            pt = ptp.tile([128, 1024], BF16, tag="pt",
                                          name="pt")
                            emit_exp(pt, ps)
                            if not mask_mm_:
                                for dp in range(2):
                                    si = 2 * p + dp
                                    r = si - 4 * tb
                                    if 0 <= r < 4:
                                        sl = pt[:, dp * 512 + r * 128:
                                                dp * 512 + (r + 1) * 128]
                                        nc.vector.tensor_mul(sl, sl,
                                                             mask_sb[:])
                            pts.append(pt)
                        return pts

                    def emit_pair_AV(p, pts):
                        for dp in range(2):
                            si = 2 * p + dp
                            for h in range(2):
                                r = si - 4 * tb
                                c0 = max(r, 0) * 128
                                nc.tensor.matmul(
                                    po[h][:, c0:512],
                                    v2[si][:, h * 65:(h + 1) * 65],
                                    pts[h][:, dp * 512 + c0:
                                           (dp + 1) * 512],
                                    start=(si == 0), stop=(si == slast),
                                    skip_group_check=True)

                    if sb_pipe and not skip_b:
                        # S(p+1) is emitted before AV(p) so the PE queue
                        # always has the next pair's S ready for ACT.
                        if z_defer and st["prev"] is not None:
                            emit_z(*st.pop("prev"))
                            st["prev"] = None
                        prev_pts = emit_pair_S(0)
                        for p in range(1, npairs):
                            pts = emit_pair_S(p)
                            emit_pair_AV(p - 1, prev_pts)
                            prev_pts = pts
                        emit_pair_AV(npairs - 1, prev_pts)
                        npairs = 0  # skip the plain loop below
                    mask_mm = cfg.get("mask_mm", False)
                    for p in range(npairs):
                        pts = []
                        psh = {}
                        if s_ilv:
                            # interleave h0/h1 S mms (rows 0-63 vs 64-127)
                            # so the PE row-tiles them concurrently
                            for h in range(2):
                                psh[h] = ps_s.tile([128, 1024], F32,
                                                   tag="sp", name="psh")
                            for dp in range(2):
                                si = 2 * p + dp
                                r = si - 4 * tb
                                rtrim = (1, 2, 3) if s_bf16 else (1, 2)
                                c0 = 128 * r if r in rtrim else 0
                                for h in range(2):
                                    nc.tensor.matmul(
                                        psh[h][:, dp * 512 + c0:
                                               (dp + 1) * 512],
                                        qT2[64 * h:64 * h + 64,
                                            si * 128:(si + 1) * 128],
                                        kT2[64 * h:64 * h + 64,
                                            tb * 512 + c0:(tb + 1) * 512],
                                        start=True, stop=True)
                        for h in range(2):
                            if s_ilv:
                                ps = psh[h]
                            else:
                                ps = ps_s.tile([128, 1024], F32, tag="sp")
                                for dp in range(2):
                                    si = 2 * p + dp
                                    r = si - 4 * tb
                                    rtrim = (1, 2, 3) if s_bf16 else (1, 2)
                                    c0 = 128 * r if r in rtrim else 0
                                    diag = mask_mm and 0 <= r < 4
                                    nc.tensor.matmul(
                                        ps[:, dp * 512 + c0:(dp + 1) * 512],
                                        qT2[64 * h:64 * h + 64,
                                            si * 128:(si + 1) * 128],
                                        kT2[64 * h:64 * h + 64,
                                            tb * 512 + c0:(tb + 1) * 512],
                                        start=True, stop=not diag)
                                    if diag:
                                        # accumulate -30*strict_lower_tri
                                        # into the diagonal 128-col chunk
                                        nc.tensor.matmul(
                                            ps[:, dp * 512 + r * 128:
                                               dp * 512 + (r + 1) * 128],
                                            identb[:], ntri_sb[:],
                                            start=False, stop=True,
                                            skip_group_check=True)
                            pt = ptp.tile([128, 1024], BF16, tag="pt")
                            if cfg.get("probe_exp_half"):
                                # TIMING PROBE ONLY (wrong numerics): exp
                                # half the tile to test ACT-boundedness
                                nc.scalar.activation(pt[:, 0:512],
                                                     ps[:, 0:512],
                                                     EXP, bias=ebias[:])
                            elif exp_trim and 2 * p >= 4 * tb:
                                # diagonal pair: exp only live cols per half
                                for dp in range(2):
                                    r = 2 * p + dp - 4 * tb
                                    c0 = 128 * r if r in (1, 2, 3) else 0
                                    nc.scalar.activation(
                                        pt[:, dp * 512 + c0:(dp + 1) * 512],
                                        ps[:, dp * 512 + c0:(dp + 1) * 512],
                                        EXP, bias=ebias[:])
                            else:
                                emit_exp(pt, ps)
                            for dp in range(0 if (cfg.get("no_mask")
                                                  or mask_mm) else 2):
                                si = 2 * p + dp
                                r = si - 4 * tb
                                if 0 <= r < 4:
                                    sl = pt[:, dp * 512 + r * 128:
                                            dp * 512 + (r + 1) * 128]
                                    meng = (nc.gpsimd if cfg.get("mask_pool")
                                            else nc.vector)
                                    meng.tensor_mul(sl, sl, mask_sb[:])
                            pts.append(pt)
                        for dp in range(2 if not skip_b else 0):
                            si = 2 * p + dp
                            for h in range(2):
                                r = si - 4 * tb
                                c0 = max(r, 0) * 128
                                nc.tensor.matmul(
                                    po[h][:, c0:512],
                                    v2[si][:, h * 65:(h + 1) * 65],
                                    pts[h][:, dp * 512 + c0:
                                           (dp + 1) * 512],
                                    start=(si == 0), stop=(si == slast),
                                    skip_group_check=True)
                        if z_defer and p == 0 and st["prev"] is not None:
                            emit_z(*st.pop("prev"))
                            st["prev"] = None

                    # ---- normalize (rows 1:65 / row 0) + partial z ----
                    if not skip_b and not skip_z:
                        outT = smallp.tile([128, 512],
                                           BF16 if z_bf16 else F32R,
                                           tag="outT")
                        po_copy = cfg.get("po_copy", False)
                        for h in range(2):
                            if po_copy:
                                # single fast eviction frees the po bank;
                                # normalize then runs off SBUF at leisure
                                posb = smallp.tile([65, 512], F32,
                                                   tag="posb", name="posb")
                                nc.vector.tensor_copy(posb[:], po[h][:])
                                src = posb
                            else:
                                src = po[h]
                            rrow = smallp.tile([1, 512], F32, tag="rrow")
                            nc.vector.reciprocal(rrow[:], src[64:65, :])
                            rbc = smallp.tile([64, 512], F32, tag="rbc")
                            nc.gpsimd.partition_broadcast(rbc[:], rrow[:])
                            nc.vector.tensor_mul(
                                outT[64 * h:64 * h + 64, :],
                                src[0:64, :], rbc[:])
                        if z_defer:
                            st["prev"] = (outT, b, tb)
                        else:
                            emit_z(outT, b, tb)

                if cfg.get("allfront"):
                    # ALL proj/v2 for both batches first (PE-only prologue),
                    # then all 8 attention blocks back-to-back so ACT never
                    # starves; next iteration's prologue overlaps this
                    # iteration's ACT tail through the FIFO queues.
                    st_all = {bb: {"xth": [[None] * 4 for _ in range(NEC)],
                                   "v2": [None] * NSC} for bb in range(B)}
                    cur = [None]

                    def use_b(bb):
                        if cur[0] == bb:
                            return
                        prev = st.get("prev")
                        if cur[0] is not None:
                            st_all[cur[0]] = {k: v for k, v in st.items()
                                              if k != "prev"}
                        st.clear()
                        st.update(st_all[bb])
                        st["prev"] = prev
                        cur[0] = bb

                    for bb in range(B):
                        use_b(bb)
                        for qt in range(4):
                            emit_xth(bb, qt)
                        for nm in ("q", "k", "v"):
                            for tp2 in range(2):
                                emit_proj_chunk(nm, tp2)
                        for s in range(NSC):
                            emit_v2(s)
                    for bb in range(B):
                        use_b(bb)
                        for tb in range(NTB if not skip_attn else 0):
                            emit_stageB(bb, tb)
                elif cfg.get("pipe2"):
                    # explicit cross-batch software pipeline: b=1 proj blocks
                    # are emitted between b=0's heavy attention blocks so the
                    # PE has work while ACT chews on exp.
                    st_all = {bb: {"xth": [[None] * 4 for _ in range(NEC)],
                                   "v2": [None] * NSC} for bb in range(B)}
                    cur = [None]

                    def use_b(bb):
                        if cur[0] == bb:
                            return
                        prev = st.get("prev")
                        if cur[0] is not None:
                            st_all[cur[0]] = {k: v for k, v in st.items()
                                              if k != "prev"}
                        st.clear()
                        st.update(st_all[bb])
                        st["prev"] = prev
                        cur[0] = bb

                    seq = [("x", 0, 0), ("x", 0, 1), ("P", 0, 0),
                           ("V", 0, 0), ("A", 0, 0), ("A", 0, 1),
                           ("x", 0, 2), ("x", 0, 3), ("P", 0, 1),
                           ("V", 0, 1), ("A", 0, 2),
                           ("x", 1, 0), ("x", 1, 1), ("P", 1, 0),
                           ("V", 1, 0), ("A", 0, 3),
                           ("A", 1, 0), ("A", 1, 1),
                           ("x", 1, 2), ("x", 1, 3), ("P", 1, 1),
                           ("V", 1, 1), ("A", 1, 2), ("A", 1, 3)]
                    for op, bb, i in seq:
                        use_b(bb)
                        if op == "x":
                            emit_xth(bb, i)
                        elif op == "P":
                            for nm in ("q", "k", "v"):
                                emit_proj_chunk(nm, i)
                        elif op == "V":
                            for s in range(8 * i, 8 * i + 8):
                                emit_v2(s)
                        else:
                            emit_stageB(bb, i)
                else:
                    for b in range(B):
                        st["xth"] = [[None] * 4 for _ in range(NEC)]
                        st["v2"] = [None] * NSC
                        if interleave:
                            for tp2 in range(2):
                                for qt in (2 * tp2, 2 * tp2 + 1):
                                    emit_xth(b, qt)
                                for nm in ("q", "k", "v"):
                                    emit_proj_chunk(nm, tp2)
                                for s in range(8 * tp2, 8 * tp2 + 8):
                                    emit_v2(s)
                                if not skip_attn:
                                    emit_stageB(b, 2 * tp2)
                                    emit_stageB(b, 2 * tp2 + 1)
                        else:
                            for qt in range(4):
                                emit_xth(b, qt)
                            for nm in ("q", "k", "v"):
                                for tp2 in range(2):
                                    emit_proj_chunk(nm, tp2)
                            for s in range(NSC):
                                emit_v2(s)
                            for tb in range(NTB if not skip_attn else 0):
                                emit_stageB(b, tb)
                if z_defer and st["prev"] is not None:
                    emit_z(*st["prev"])

            if rep == 1:
                body()
            elif cfg.get("unroll"):
                for _ in range(rep):
                    body()
            elif cfg.get("body2") and (rep - 1) % 2 == 0:
                if cfg.get("sreset"):
                    with tc.For_i(0, (rep - 1) // 2, 1,
                                  staggered_reset=True):
                        body()
                        body()
                else:
                    with tc.For_i(0, (rep - 1) // 2, 1):
                        body()
                        body()
                body()
            elif cfg.get("sreset"):
                with tc.For_i(0, rep, 1, staggered_reset=True):
                    body()
            else:
                with tc.For_i(0, rep, 1):
                    body()

    nc.compile()
    return nc


def build_nc_bh(rep=1, cfg=None):
    """Batch x head sharding: core c owns batch c//4 and heads
    4*(c%4)..4*(c%4)+3 (two pairs hp=0,1). All-bf16 matmul path.
    z partial accumulates both pairs in PSUM; host sums 4 cores/batch."""
    cfg = dict(cfg or {})
    sp_bufs = cfg.get("sp_bufs", 2)
    op_bufs = cfg.get("op_bufs", 2)
    pt_bufs = cfg.get("pt_bufs", 4)
    out_q = cfg.get("out_q", "sync")
    exp_trim = cfg.get("exp_trim", True)
    zevict = cfg.get("zevict", "any")  # any|vector|scalar|pool
    NHP = 2                     # head pairs per core
    nc = bacc.Bacc("TRN2", target_bir_lowering=False, debug=False,
                   num_devices=N_CORES)

    xt = nc.dram_tensor("xt", [E, T], BF16, kind="ExternalInput").ap()
    wq = nc.dram_tensor("wq", [E, NHP * F], BF16, kind="ExternalInput").ap()
    wk = nc.dram_tensor("wk", [E, NHP * F], BF16, kind="ExternalInput").ap()
    wv = nc.dram_tensor("wv", [E, NHP * F], BF16, kind="ExternalInput").ap()
    wot = nc.dram_tensor("wot", [NHP * F, E], BF16,
                         kind="ExternalInput").ap()
    mask = nc.dram_tensor("mask", [128, 128], BF16, kind="ExternalInput").ap()
    zp = nc.dram_tensor("zp", [T, E], F16, kind="ExternalOutput").ap()

    with tile.TileContext(nc) as tc:
        with (
            tc.tile_pool(name="const", bufs=1) as cpool,
            tc.tile_pool(name="xtp", bufs=36) as xtp,
            tc.tile_pool(name="proj", bufs=2) as projp,
            tc.tile_pool(name="v2p", bufs=2 * NSC + 8) as v2p,
            tc.tile_pool(name="ptp", bufs=pt_bufs) as ptp,
            tc.tile_pool(name="outTp",
                         bufs=(5 if cfg.get("z_defer") else 3)) as outTp,
            tc.tile_pool(name="smallp", bufs=4) as smallp,
            tc.tile_pool(name="zsbp", bufs=3) as zsbp,
            tc.tile_pool(name="ps_s", bufs=sp_bufs, space="PSUM") as ps_s,
            tc.tile_pool(name="ps_o", bufs=op_bufs, space="PSUM") as ps_o,
            tc.tile_pool(name="ps_t", bufs=2, space="PSUM") as ps_t,
        ):
            # ---- constants (loaded once) ----
            ident = cpool.tile([128, 128], F32, tag="ident")
            make_identity(nc, ident[:])
            mask_sb = cpool.tile([128, 128], BF16, tag="mask")
            nc.scalar.dma_start(mask_sb[:], mask)
            ebias = cpool.tile([128, 1], F32, tag="ebias")
            nc.vector.memset(ebias[:], EXP_BIAS)
            # weights: [E, 2F] -> [128, NEC*2F]; w[hp][e] = [128, F]
            wsb = {}
            for wsrc, nm in ((wq, "wq"), (wk, "wk"), (wv, "wv")):
                t_ = cpool.tile([128, NEC * NHP * F], BF16, tag=f"wall{nm}")
                nc.scalar.dma_start(
                    t_.rearrange("p (a c) -> p a c", a=NEC),
                    wsrc.rearrange("(a p) c -> p a c", p=128))
                wsb[nm] = [[t_[:, e * NHP * F + hp * F:
                               e * NHP * F + (hp + 1) * F]
                            for e in range(NEC)] for hp in range(NHP)]
            wot_sb = []
            for hp in range(NHP):
                t_ = cpool.tile([F, E], BF16, tag=f"wot{hp}")
                nc.scalar.dma_start(t_[:], wot[hp * F:(hp + 1) * F, :])
                wot_sb.append(t_)

            def body():
                xth = [[None] * 4 for _ in range(NEC)]
                qT2, kT2, vT2, v2 = {}, {}, {}, {hp: [None] * NSC
                                                 for hp in range(NHP)}
                z_defer = cfg.get("z_defer", False)
                interleave = cfg.get("interleave", False)

                def emit_xth(qt):
                    for e in range(NEC):
                        t_ = xtp.tile([128, T // 4], BF16, tag="xt")
                        eng = ((nc.sync, nc.scalar)[e % 2]
                               if cfg.get("xt_q") else nc.sync)
                        eng.dma_start(
                            t_[:], xt[e * 128:(e + 1) * 128,
                                      qt * (T // 4):(qt + 1) * (T // 4)])
                        xth[e][qt] = t_

                def emit_proj_chunk(hp, nm, tp2):
                    if tp2 == 0:
                        pdt = F32 if nm == "v" else BF16
                        dst = projp.tile([128, T], pdt, tag=f"{nm}T2_{hp}")
                        {"q": qT2, "k": kT2, "v": vT2}[nm][hp] = dst
                    dst = {"q": qT2, "k": kT2, "v": vT2}[nm][hp]
                    ps = ps_s.tile([128, 1024], F32, tag="sp")
                    for half in range(2):
                        c0 = tp2 * 1024 + half * 512
                        for e in range(NEC):
                            nc.tensor.matmul(
                                ps[:, half * 512:(half + 1) * 512],
                                wsb["w" + nm][hp][e],
                                xth[e][c0 // 512][:],
                                start=(e == 0), stop=(e == NEC - 1))
                    nc.any.tensor_copy(
                        dst[:, tp2 * 1024:(tp2 + 1) * 1024], ps[:])

                def emit_v2(hp, s):
                    tpw = ps_t.tile([128, 512], F32, tag="tp")
                    tp_ = tpw[:, 0:128]
                    nc.tensor.matmul(tp_[:],
                                     vT2[hp][:, s * 128:(s + 1) * 128],
                                     ident[:], is_transpose=True)
                    v2t = v2p.tile([128, 130], BF16, tag="v2")
                    v2r = v2t.rearrange("p (g c) -> p g c", g=2)
                    nc.vector.memset(v2r[:, :, 64:65], 1.0)
                    nc.vector.tensor_copy(
                        v2r[:, :, 0:64],
                        tp_.rearrange("p (g c) -> p g c", g=2))
                    v2[hp][s] = v2t

                def emit_z(zoutT, ztb):
                    for jp2 in range(2):
                        zsb = zsbp.tile([128, 2048], F16, tag="zsb")
                        for jj in range(2):
                            j = 2 * jp2 + jj
                            zpsl = []
                            for eb in range(2):
                                zps = ps_t.tile([128, 512], F32, tag="tp")
                                zpsl.append(zps)
                            for hp in range(NHP):
                                for eb in range(2):
                                    nc.tensor.matmul(
                                        zpsl[eb][:],
                                        zoutT[hp][:, j * 128:(j + 1) * 128],
                                        wot_sb[hp][:, eb * 512:(eb + 1) * 512],
                                        start=(hp == 0), stop=(hp == NHP - 1))
                            for eb in range(2):
                                dstsl = zsb[:, jj * 1024 + eb * 512:
                                            jj * 1024 + (eb + 1) * 512]
                                if zevict == "vector":
                                    nc.vector.tensor_copy(dstsl, zpsl[eb][:])
                                elif zevict == "scalar":
                                    nc.scalar.copy(dstsl, zpsl[eb][:])
                                else:
                                    nc.any.tensor_copy(dstsl, zpsl[eb][:])
                        t0r = (4 * ztb + 2 * jp2) * 128
                        getattr(nc, out_q).dma_start(
                            zp[t0r:t0r + 256, :]
                            .rearrange("(a p) c -> p a c", p=128),
                            zsb.rearrange("p (a c) -> p a c", a=2))

                prev_outT = None

                def emit_stageB(tb):
                    nonlocal prev_outT
                    slast = 4 * tb + 3
                    outT = {}
                    for hp in range(NHP):
                        po = {}
                        for h in range(2):
                            po_t = ps_o.tile([65, 512], F32, tag="op")
                            po[h] = po_t
                        npairs = 2 * tb + 2
                        for p in range(npairs):
                            pts = []
                            for h in range(2):
                                ps = ps_s.tile([128, 1024], F32, tag="sp")
                                for dp in range(2):
                                    si = 2 * p + dp
                                    r = si - 4 * tb
                                    c0 = 128 * r if r in (1, 2, 3) else 0
                                    nc.tensor.matmul(
                                        ps[:, dp * 512 + c0:(dp + 1) * 512],
                                        qT2[hp][64 * h:64 * h + 64,
                                                si * 128:(si + 1) * 128],
                                        kT2[hp][64 * h:64 * h + 64,
                                                tb * 512 + c0:(tb + 1) * 512],
                                        start=True, stop=True)
                                pt = ptp.tile([128, 1024], BF16, tag="pt")
                                if exp_trim and 2 * p >= 4 * tb:
                                    # diagonal pair: exp live cols per half
                                    for dp in range(2):
                                        r = 2 * p + dp - 4 * tb
                                        c0 = 128 * r if r in (1, 2, 3) else 0
                                        nc.scalar.activation(
                                            pt[:, dp * 512 + c0:
                                               (dp + 1) * 512],
                                            ps[:, dp * 512 + c0:
                                               (dp + 1) * 512],
                                            EXP, bias=ebias[:])
                                else:
                                    nc.scalar.activation(pt[:], ps[:], EXP,
                                                         bias=ebias[:])
                                for dp in range(2):
                                    si = 2 * p + dp
                                    r = si - 4 * tb
                                    if 0 <= r < 4:
                                        sl = pt[:, dp * 512 + r * 128:
                                                dp * 512 + (r + 1) * 128]
                                        nc.vector.tensor_mul(sl, sl,
                                                             mask_sb[:])
                                pts.append(pt)
                            for dp in range(2):
                                si = 2 * p + dp
                                for h in range(2):
                                    r = si - 4 * tb
                                    c0 = max(r, 0) * 128
                                    nc.tensor.matmul(
                                        po[h][:, c0:512],
                                        v2[hp][si][:, h * 65:(h + 1) * 65],
                                        pts[h][:, dp * 512 + c0:
                                               (dp + 1) * 512],
                                        start=(si == 0), stop=(si == slast),
                                        skip_group_check=True)
                            # deferred z of the previous t-block slots into
                            # the PE queue here, after deps are long ready
                            if z_defer and hp == 0 and p == 0 and tb > 0:
                                emit_z(prev_outT, tb - 1)

                        # ---- normalize (rows 0:64 / row 64) ----
                        oT = outTp.tile([128, 512], BF16, tag="outT")
                        for h in range(2):
                            rrow = smallp.tile([1, 512], F32, tag="rrow")
                            nc.vector.reciprocal(rrow[:], po[h][64:65, :])
                            rbc = smallp.tile([64, 512], F32, tag="rbc")
                            nc.gpsimd.partition_broadcast(rbc[:], rrow[:])
                            nc.vector.tensor_mul(
                                oT[64 * h:64 * h + 64, :],
                                po[h][0:64, :], rbc[:])
                        outT[hp] = oT

                    # ---- z: accumulate both pairs in PSUM ----
                    if z_defer:
                        prev_outT = outT
                    else:
                        emit_z(outT, tb)

                if interleave:
                    for tp2 in range(2):
                        for qt in (2 * tp2, 2 * tp2 + 1):
                            emit_xth(qt)
                        for hp in range(NHP):
                            for nm in ("q", "k", "v"):
                                emit_proj_chunk(hp, nm, tp2)
                        for hp in range(NHP):
                            for s in range(8 * tp2, 8 * tp2 + 8):
                                emit_v2(hp, s)
                        emit_stageB(2 * tp2)
                        emit_stageB(2 * tp2 + 1)
                else:
                    for qt in range(4):
                        emit_xth(qt)
                    for hp in range(NHP):
                        for nm in ("q", "k", "v"):
                            for tp2 in range(2):
                                emit_proj_chunk(hp, nm, tp2)
                        for s in range(NSC):
                            emit_v2(hp, s)
                    for tb in range(NTB):
                        emit_stageB(tb)
                if z_defer and prev_outT is not None:
                    emit_z(prev_outT, NTB - 1)

            if rep == 1:
                body()
            elif cfg.get("unroll"):
                for _ in range(rep):
                    body()
            elif cfg.get("body2") and (rep - 1) % 2 == 0:
                with tc.For_i(0, (rep - 1) // 2, 1):
                    body()
                    body()
                body()
            elif cfg.get("sreset"):
                with tc.For_i(0, rep, 1, staggered_reset=True):
                    body()
            else:
                with tc.For_i(0, rep, 1):
                    body()

    nc.compile()
    return nc


def make_in_maps_bh(inputs, Wk, Wq, Wv, Wo):
    """Shard: core c gets batch c//4, heads 4*(c%4)..4*(c%4)+3."""
    bf = ml_dtypes.bfloat16
    scale = np.float32(D ** -0.5)
    tri = (np.arange(128)[None, :] >= np.arange(128)[:, None])
    mask = tri.astype(bf)
    in_maps = []
    for c in range(N_CORES):
        b = c // 4
        h0 = 4 * (c % 4)
        xt = np.ascontiguousarray(inputs[b].T).astype(bf)
        wq2 = np.concatenate([Wq[h0 + i] for i in range(4)], axis=1)
        wk2 = np.concatenate([Wk[h0 + i] for i in range(4)], axis=1) * scale
        wv2 = np.concatenate([Wv[h0 + i] for i in range(4)], axis=1)
        wot = np.ascontiguousarray(Wo[:, 64 * h0:64 * (h0 + 4)].T)
        in_maps.append({
            "xt": xt,
            "wq": np.ascontiguousarray(wq2).astype(bf),
            "wk": np.ascontiguousarray(wk2).astype(bf),
            "wv": np.ascontiguousarray(wv2).astype(bf),
            "wot": wot.astype(bf),
            "mask": mask,
        })
    return in_maps


def make_in_maps(inputs, Wk, Wq, Wv, Wo, xt_bf16=False, z_bf16=False):
    """Shard full inputs into per-core input maps."""
    wdt = ml_dtypes.bfloat16 if xt_bf16 else np.float32
    zdt = ml_dtypes.bfloat16 if z_bf16 else np.float32
    xt = np.ascontiguousarray(inputs.transpose(0, 2, 1)).astype(wdt)
    scale = np.float32(D ** -0.5)
    tri = (np.arange(128)[None, :] >= np.arange(128)[:, None])
    mask = tri.astype(ml_dtypes.bfloat16)
    # -30 on strictly-lower (k > j): masks S[s,t] where s > t via matmul
    ntri = (-30.0 * (np.arange(128)[:, None] > np.arange(128)[None, :])
            ).astype(ml_dtypes.bfloat16)
    in_maps = []
    for c in range(N_CORES):
        h0 = HPC * c
        wq2 = np.ascontiguousarray(
            np.concatenate([Wq[h0 + i] for i in range(HPC)], axis=1))
        wk2 = np.ascontiguousarray(
            np.concatenate([Wk[h0 + i] for i in range(HPC)], axis=1)) * scale
        wv2 = np.ascontiguousarray(
            np.concatenate([Wv[h0 + i] for i in range(HPC)], axis=1))
        wot = np.ascontiguousarray(Wo[:, F * c:F * (c + 1)].T)
        in_maps.append({
            "xt": xt,
            "wq": wq2.astype(wdt),
            "wk": wk2.astype(wdt),
            "wv": wv2.astype(wdt),
            "wot": wot.astype(zdt),
            "mask": mask,
            "ntri": ntri,
        })
    return in_maps


_NC = None
_RUN = None
DEFAULT_CFG = {"any_copy": True, "out_q": "sync", "xt_bf16": True,
               "s_bf16": True, "z_bf16": True,
               "interleave": True, "z_defer": True, "sreset": True,
               "body2": True}


def combine(zp_list, bo, cfg):
    """Combine per-core zp partials into the full [B, T, E] output."""
    z = np.zeros((B, T, E), dtype=np.float32)
    if cfg.get("bh"):
        for c in range(N_CORES):
            z[c // 4] += np.asarray(zp_list[c]).astype(np.float32)
    else:
        for c in range(N_CORES):
            z += np.asarray(zp_list[c]).astype(np.float32)
    return z + bo.astype(np.float32)


def kernel(inputs, Wk, Wq, Wv, Wo, bo):
    global _NC, _RUN
    if _NC is None:
        _NC = build_nc(cfg=DEFAULT_CFG)
    if DEFAULT_CFG.get("bh"):
        in_maps = make_in_maps_bh(inputs, Wk, Wq, Wv, Wo)
    else:
        in_maps = make_in_maps(inputs, Wk, Wq, Wv, Wo,
                               xt_bf16=DEFAULT_CFG.get("xt_bf16", False),
                               z_bf16=DEFAULT_CFG.get("z_bf16", False))
    # drop inputs the built kernel does not declare (e.g. ntri w/o mask_mm)
    declared = {
        a.memorylocations[0].name
        for a in _NC.m.functions[0].allocations
        if isinstance(a, mybir.MemoryLocationSet) and a.kind == "ExternalInput"
    }
    in_maps = [{k: v for k, v in m.items() if k in declared}
               for m in in_maps]
    try:
        if _RUN is None:
            _RUN = _make_runner(_NC)
        results = _RUN(in_maps)
    except Exception:
        _RUN = False if _RUN is None else _RUN
        res = run_bass_kernel_spmd(_NC, in_maps,
                                   core_ids=list(range(N_CORES)))
        results = res.results
    return combine([results[c]["zp"] for c in range(N_CORES)], bo,
                   DEFAULT_CFG)

